# revision 1
# baseline (speedup 1.0000x reference)
"""CrossAttentionBlock on 8 trn2 NeuronCores.

Sharding (per the hint): data parallel over batch B=2, tensor parallel over
heads (16 heads -> 4 groups of 4). Core c = b*4 + hg.

Phase A (8 cores): per (b, head-group) compute q/k/v projections for the
group's 256 dims, then masked softmax(QK^T/sqrt(d))V per head, normalized.
Everything is kept transposed ([dim x seq]) so the tensor engine contracts
along partitions at every step:
  - scores^T[kv, q] = kT[d, kv].T @ qT[d, q]           (K=64 per head)
  - exp evacuates scores PSUM->SBUF in one ACT pass per tile
  - mask folded multiplicatively into V (and into the denominator via a
    mask column appended to V), so exp needs no bias and no row max:
    scores are N(0,1)-scale so exp never overflows fp32.
  - attnV^T + denominator in one matmul per kv-tile via the [v | mask]
    65-column stationary operand.
Output: attnT (256 x 1024) per core, already normalized.

Phase B (8 cores): rows sharded (256 rows of B*SQ each): out = attn @ Wo.T
+ bo + residual, then LayerNorm. attnT from phase A is exactly the lhsT the
out-projection needs.

All matmuls run as float32r (full-rate fp32 on the PE; ~1e-4 rel err
measured on HW vs fp64-exact numpy).
"""

import numpy as np
from contextlib import ExitStack

import concourse.bacc as bacc
import concourse.tile as tile
import concourse.mybir as mybir
from concourse.bass_utils import run_bass_kernel_spmd

F32 = mybir.dt.float32
F32R = mybir.dt.float32r
AF = mybir.ActivationFunctionType
ALU = mybir.AluOpType
AX = mybir.AxisListType

B, SQ, SKV, E = 2, 1024, 4096, 1024
H, D = 16, 64
HG = 4                 # heads per core
HD = HG * D            # 256
P = 128
NE = E // P            # 8
NKV = SKV // P         # 32
LN_EPS = 1e-5
SCALE = 1.0 / np.sqrt(D)

_CACHE = {}


def _build_phase_a(ck_bufs=2, ex_bufs=4, sc_bufs=4, at_bufs=2, pj_bufs=2, n_streams=4, sc2_bufs=3, sm_bufs=2, n_proj=None):
    nc = bacc.Bacc("TRN2", target_bir_lowering=False, debug=False, num_devices=8)

    qT_d = nc.dram_tensor("qT", [E, SQ], F32R, kind="ExternalInput")
    kvT_d = nc.dram_tensor("kvT", [E, SKV], F32R, kind="ExternalInput")
    wqT_d = nc.dram_tensor("wqT", [P, NE * HD], F32R, kind="ExternalInput")
    wkT_d = nc.dram_tensor("wkT", [P, NE * HD], F32R, kind="ExternalInput")
    wvT_d = nc.dram_tensor("wvT", [P, NE * HD], F32R, kind="ExternalInput")
    bq_d = nc.dram_tensor("bq", [1, HD], F32R, kind="ExternalInput")
    bk_d = nc.dram_tensor("bk", [1, HD], F32R, kind="ExternalInput")
    bv_d = nc.dram_tensor("bv", [1, HD], F32R, kind="ExternalInput")
    mask_d = nc.dram_tensor("mask01", [P, NKV], F32, kind="ExternalInput")
    attnT_d = nc.dram_tensor("attnT", [HD, SQ], F32, kind="ExternalOutput")

    with tile.TileContext(nc) as tc, ExitStack() as ctx:
        const = ctx.enter_context(tc.tile_pool(name="const", bufs=1))

        wq_sb = const.tile([P, NE, HD], F32R)
        nc.sync.dma_start(wq_sb[:], wqT_d.ap().rearrange("p (j d) -> p j d", d=HD))
        bq_sb = const.tile([1, HD], F32R)
        nc.sync.dma_start(bq_sb[:], bq_d.ap())
        wk_sb = const.tile([P, NE, HD], F32R)
        nc.sync.dma_start(wk_sb[:], wkT_d.ap().rearrange("p (j d) -> p j d", d=HD))
        wv_sb = const.tile([P, NE, HD], F32R)
        nc.sync.dma_start(wv_sb[:], wvT_d.ap().rearrange("p (j d) -> p j d", d=HD))
        bk_sb = const.tile([1, HD], F32R)
        nc.sync.dma_start(bk_sb[:], bk_d.ap())
        bv_sb = const.tile([1, HD], F32R)
        nc.sync.dma_start(bv_sb[:], bv_d.ap())
        mask_sb = const.tile([P, NKV], F32)
        nc.sync.dma_start(mask_sb[:], mask_d.ap())
        ones32_sb = const.tile([1, 512], F32)
        nc.any.memset(ones32_sb[:], 1.0)
        ones_sb = const.tile([1, 512], F32R)
        nc.vector.tensor_copy(ones_sb[:], ones32_sb[:])

        # per-chunk product tiles: attention on kv-tile t depends only on
        # chunk t//4's tiles, so it can overlap later projection chunks
        # (one big tile would serialize attention behind the last chunk).
        qTs_sb = const.tile([P, 2, SQ], F32R)
        attnT_sb = const.tile([P, 2, SQ], F32)
        kT_c = [const.tile([P, 2, 512], F32R, name=f"kTc{c}") for c in range(8)]
        v_c = [const.tile([P, 4, HG * (D + 1)], F32R, name=f"vc{c}") for c in range(8)]

        # mask columns of v_aug: v[:, tt, 65*h + 64] = mask[:, 4c+tt]
        maskr_sb = const.tile([P, NKV], F32R)
        nc.vector.tensor_copy(maskr_sb[:], mask_sb[:])
        for c in range(8):
            for h in range(HG):
                nc.vector.tensor_copy(v_c[c][:, :, h * (D + 1) + D],
                                      maskr_sb[:, 4 * c:4 * c + 4])

        # ---------------- pools. Two regions:
        #  R1: projections with attention stream (0,0) interleaved.
        #      PSUM: accum 2 + scores512 x4 = 4 + proj 2 = 8 banks.
        #  R2: attention streams (0,1),(1,0),(1,1), 1024-wide scores tiles.
        #      PSUM: accum 2 + scores1024 x3 = 6 -> 8 banks.
        at_ps = ctx.enter_context(tc.tile_pool(name="atps", bufs=at_bufs, space="PSUM"))
        ex_pool = ctx.enter_context(tc.tile_pool(name="expool", bufs=ex_bufs))
        sm_pool = ctx.enter_context(tc.tile_pool(name="smpool", bufs=sm_bufs))

        def do_q_chunk(c, ck_pool, pj_ps):
            ch = ck_pool.tile([P, NE, 512], F32R, tag="ch", name=f"chq{c}")
            for j in range(NE):
                nc.sync.dma_start(ch[:, j, :], qT_d.ap()[j * P:(j + 1) * P, c * 512:(c + 1) * 512])
            for m in range(2):
                ps = pj_ps.tile([P, 512], F32, tag="pj", name=f"qps{c}_{m}")
                for j in range(NE):
                    nc.tensor.matmul(ps[:], wq_sb[:, j, m * P:(m + 1) * P], ch[:, j, :],
                                     start=(j == 0), stop=False)
                nc.tensor.matmul(ps[:], bq_sb[:, m * P:(m + 1) * P], ones_sb[:],
                                 start=False, stop=True)
                nc.vector.tensor_copy(qTs_sb[:, m, c * 512:(c + 1) * 512], ps[:])

        def do_kv_chunk(c, ck_pool, pj_ps):
            ch = ck_pool.tile([P, NE, 512], F32R, tag="ch", name=f"chkv{c}")
            for j in range(NE):
                nc.sync.dma_start(ch[:, j, :], kvT_d.ap()[j * P:(j + 1) * P, c * 512:(c + 1) * 512])
            for m in range(2):
                ps = pj_ps.tile([P, 512], F32, tag="pj", name=f"kps{c}_{m}")
                for j in range(NE):
                    nc.tensor.matmul(ps[:], wk_sb[:, j, m * P:(m + 1) * P], ch[:, j, :],
                                     start=(j == 0), stop=False)
                nc.tensor.matmul(ps[:], bk_sb[:, m * P:(m + 1) * P], ones_sb[:],
                                 start=False, stop=True)
                nc.vector.tensor_copy(kT_c[c][:, m, :], ps[:])
            for tt in range(4):
                t = 4 * c + tt
                ps = pj_ps.tile([P, HD], F32, tag="pj", name=f"vps{t}")
                for j in range(NE):
                    nc.tensor.matmul(ps[:], ch[:, j, tt * P:(tt + 1) * P], wv_sb[:, j, :],
                                     start=(j == 0), stop=False)
                nc.tensor.matmul(ps[:], ones_sb[:, 0:P], bv_sb[:], start=False, stop=True)
                nc.vector.tensor_scalar(
                    v_c[c][:, tt, :].rearrange("p (h u) -> p h u", u=D + 1)[:, :, 0:D],
                    ps[:].rearrange("p (h u) -> p h u", u=D),
                    mask_sb[:, t:t + 1], None, op0=ALU.mult)

        def att_scores_512(sc_pool, m, qh, t, hh):
            ps = sc_pool.tile([P, 512], F32, tag="sc", name=f"s{m}{qh}_{t}_{hh}")
            nc.tensor.matmul(
                ps[:],
                kT_c[t // 4][hh * D:(hh + 1) * D, m, (t % 4) * P:(t % 4 + 1) * P],
                qTs_sb[hh * D:(hh + 1) * D, m, qh * 512:(qh + 1) * 512],
                start=True, stop=True)
            ex = ex_pool.tile([P, 512], F32R, tag="ex", name=f"e{m}{qh}_{t}_{hh}")
            nc.scalar.activation(ex[:], ps[:], AF.Exp, scale=float(SCALE))
            return ex

        def att_scores_1024(sc_pool, m, qh, t):
            ps = sc_pool.tile([P, 1024], F32, tag="sc", name=f"s{m}{qh}_{t}")
            for hh in range(2):
                nc.tensor.matmul(
                    ps[:, hh * 512:(hh + 1) * 512],
                    kT_c[t // 4][hh * D:(hh + 1) * D, m, (t % 4) * P:(t % 4 + 1) * P],
                    qTs_sb[hh * D:(hh + 1) * D, m, qh * 512:(qh + 1) * 512],
                    start=True, stop=True)
            ex = ex_pool.tile([P, 1024], F32R, tag="ex", name=f"e{m}{qh}_{t}")
            nc.scalar.activation(ex[:], ps[:], AF.Exp, scale=float(SCALE))
            return ex

        def att_av(m, qh, t, ex, exoff, hh, pv):
            h = 2 * m + hh
            nc.tensor.matmul(
                pv[hh][:],
                v_c[t // 4][:, t % 4, h * (D + 1):(h + 1) * (D + 1)],
                ex[:, exoff:exoff + 512],
                start=(t == 0), stop=(t == NKV - 1))

        def att_norm(sc_pool, m, qh, pv):
            for hh in range(2):
                den = sm_pool.tile([1, 512], F32, tag="den", name=f"den{m}{qh}{hh}")
                nc.vector.tensor_copy(den[:], pv[hh][D:D + 1, :])
                rec = sm_pool.tile([1, 512], F32R, tag="rec", name=f"rec{m}{qh}{hh}")
                with nc.allow_low_precision(reason="recip feeds f32r matmul; PE rounds inputs anyway"):
                    nc.vector.reciprocal(rec[:], den[:])
                raw = sm_pool.tile([D, 512], F32, tag="raw", name=f"raw{m}{qh}{hh}")
                nc.vector.tensor_copy(raw[:], pv[hh][0:D, :])
                bc = sc_pool.tile([D, 512], F32, tag="sc", name=f"bc{m}{qh}{hh}")
                nc.tensor.matmul(bc[:], ones_sb[:, 0:D], rec[:], start=True, stop=True)
                nc.vector.tensor_tensor(
                    attnT_sb[hh * D:(hh + 1) * D, m, qh * 512:(qh + 1) * 512],
                    raw[:], bc[:], op=ALU.mult)

        def new_pv(m, qh):
            return [at_ps.tile([D + 1, 512], F32, tag="acc", name=f"pv{m}{qh}_{hh}")
                    for hh in range(2)]

        # ---- region 1: projections with streams (0,0),(0,1) interleaved.
        # 1-(tile,head) emission skew keeps the PE ahead of exp.
        with ExitStack() as r1:
            sc_r1 = r1.enter_context(tc.tile_pool(name="scr1", bufs=sc_bufs, space="PSUM"))
            ck_pool = r1.enter_context(tc.tile_pool(name="ck", bufs=ck_bufs))
            pj_ps = r1.enter_context(tc.tile_pool(name="pjps", bufs=pj_bufs, space="PSUM"))

            do_q_chunk(0, ck_pool, pj_ps)
            do_q_chunk(1, ck_pool, pj_ps)
            pv0 = new_pv(0, 0)
            pending = None
            for c in range(SKV // 512):
                do_kv_chunk(c, ck_pool, pj_ps)
                if n_streams == 0:
                    continue
                for t in range(4 * c, 4 * c + 4):
                    for hh in range(2):
                        ex = att_scores_512(sc_r1, 0, 0, t, hh)
                        if pending is not None:
                            att_av(0, *pending)
                        pending = (0, t, ex, 0, hh, pv0)
            if n_streams > 0:
                att_av(0, *pending)
                att_norm(sc_r1, 0, 0, pv0)

        # ---- region 2: remaining streams, attention only, 1024-wide scores
        if n_streams > 1:
            with ExitStack() as r2:
                sc_r2 = r2.enter_context(tc.tile_pool(name="scr2", bufs=sc2_bufs, space="PSUM"))
                for (m, qh) in ((0, 1), (1, 0), (1, 1))[:n_streams - 1]:
                    pv = new_pv(m, qh)
                    pending = None
                    for t in range(NKV):
                        ex = att_scores_1024(sc_r2, m, qh, t)
                        if pending is not None:
                            att_av(m, *pending)
                            att_av(m, *pending2)
                        pending = (qh, t, ex, 0, 0, pv)
                        pending2 = (qh, t, ex, 512, 1, pv)
                    att_av(m, *pending)
                    att_av(m, *pending2)
                    att_norm(sc_r2, m, qh, pv)

        nc.sync.dma_start(attnT_d.ap().rearrange("(m p) q -> p m q", p=P), attnT_sb[:])

    nc.compile()
    return nc


def _build_phase_b():
    R = 2 * P   # 256 rows per core
    nc = bacc.Bacc("TRN2", target_bir_lowering=False, debug=False, num_devices=8)

    aT_d = nc.dram_tensor("aT", [E, R], F32R, kind="ExternalInput")
    woT_d = nc.dram_tensor("woT", [E, E], F32R, kind="ExternalInput")
    qn_d = nc.dram_tensor("qn", [R, E], F32, kind="ExternalInput")
    bo_d = nc.dram_tensor("bo", [1, E], F32R, kind="ExternalInput")
    gam_d = nc.dram_tensor("gam", [1, E], F32R, kind="ExternalInput")
    bet_d = nc.dram_tensor("bet", [1, E], F32R, kind="ExternalInput")
    y_d = nc.dram_tensor("y", [R, E], F32, kind="ExternalOutput")

    with tile.TileContext(nc) as tc, ExitStack() as ctx:
        const = ctx.enter_context(tc.tile_pool(name="const", bufs=1))
        aT_sb = const.tile([P, NE, R], F32R)
        for k in range(NE):
            nc.sync.dma_start(aT_sb[:, k, :], aT_d.ap()[k * P:(k + 1) * P, :])
        qn_sb = const.tile([P, 2, E], F32)
        for mt in range(2):
            nc.sync.dma_start(qn_sb[:, mt, :], qn_d.ap().rearrange("(m p) e -> p m e", p=P)[:, mt, :])
        bo_sb = const.tile([1, E], F32R)
        nc.sync.dma_start(bo_sb[:], bo_d.ap())
        gam_sb = const.tile([1, E], F32R)
        nc.sync.dma_start(gam_sb[:], gam_d.ap())
        bet_sb = const.tile([1, E], F32R)
        nc.sync.dma_start(bet_sb[:], bet_d.ap())
        ones32_sb = const.tile([1, P], F32)
        nc.any.memset(ones32_sb[:], 1.0)
        ones_sb = const.tile([1, P], F32R)
        nc.vector.tensor_copy(ones_sb[:], ones32_sb[:])

        gam_bc = const.tile([P, E], F32)
        bet_bc = const.tile([P, E], F32)

        wo_pool = ctx.enter_context(tc.tile_pool(name="wo", bufs=4))
        ps_pool = ctx.enter_context(tc.tile_pool(name="ps", bufs=2, space="PSUM"))
        gb_ps = ctx.enter_context(tc.tile_pool(name="gbps", bufs=2, space="PSUM"))
        sbp = ctx.enter_context(tc.tile_pool(name="sbp", bufs=2))

        ps_tiles = {}
        for k in range(NE):
            wo = wo_pool.tile([P, E], F32R, tag="wo", name=f"wo{k}")
            nc.sync.dma_start(wo[:], woT_d.ap()[k * P:(k + 1) * P, :])
            for mt in range(2):
                if k == 0:
                    ps_tiles[mt] = ps_pool.tile([P, E], F32, tag="o", name=f"o{mt}")
                for nh in range(2):
                    nc.tensor.matmul(ps_tiles[mt][:, nh * 512:(nh + 1) * 512],
                                     aT_sb[:, k, mt * P:(mt + 1) * P],
                                     wo[:, nh * 512:(nh + 1) * 512],
                                     start=(k == 0), stop=False)
        for mt in range(2):
            for nh in range(2):
                nc.tensor.matmul(ps_tiles[mt][:, nh * 512:(nh + 1) * 512], ones_sb[:],
                                 bo_sb[:, nh * 512:(nh + 1) * 512],
                                 start=False, stop=True)

        # broadcast gamma/beta rows to all 128 partitions via K=1 matmuls
        for half in range(2):
            cs = slice(half * 512, (half + 1) * 512)
            psg = gb_ps.tile([P, 512], F32, tag="gb", name=f"gbg{half}")
            nc.tensor.matmul(psg[:], ones_sb[:], gam_sb[:, cs], start=True, stop=True)
            nc.scalar.copy(gam_bc[:, cs], psg[:])
            psb = gb_ps.tile([P, 512], F32, tag="gb", name=f"gbb{half}")
            nc.tensor.matmul(psb[:], ones_sb[:], bet_sb[:, cs], start=True, stop=True)
            nc.scalar.copy(bet_bc[:, cs], psb[:])

        # residual + LayerNorm, var = E[x^2] - mean^2 so the two reductions
        # run on different engines (DVE reduce, ACT Square+accum) in parallel
        for mt in range(2):
            x = sbp.tile([P, E], F32, tag="x", name=f"x{mt}")
            nc.vector.tensor_tensor(x[:], ps_tiles[mt][:], qn_sb[:, mt, :], op=ALU.add)
            s1 = sbp.tile([P, 1], F32, tag="s1", name=f"s1{mt}")
            nc.vector.reduce_sum(s1[:], x[:], axis=AX.X)
            sq = sbp.tile([P, E], F32, tag="sq", name=f"sq{mt}")
            ssq = sbp.tile([P, 1], F32, tag="ssq", name=f"ssq{mt}")
            nc.scalar.activation(sq[:], x[:], AF.Square, accum_out=ssq[:])
            nm = sbp.tile([P, 1], F32, tag="nm", name=f"nm{mt}")
            nc.vector.tensor_scalar(nm[:], s1[:], -1.0 / E, None, op0=ALU.mult)
            m2 = sbp.tile([P, 1], F32, tag="m2", name=f"m2{mt}")
            nc.vector.tensor_tensor(m2[:], nm[:], nm[:], op=ALU.mult)
            var = sbp.tile([P, 1], F32, tag="var", name=f"var{mt}")
            nc.vector.tensor_scalar(var[:], ssq[:], 1.0 / E, LN_EPS, op0=ALU.mult, op1=ALU.add)
            nc.vector.tensor_tensor(var[:], var[:], m2[:], op=ALU.subtract)
            rv = sbp.tile([P, 1], F32, tag="rv", name=f"rv{mt}")
            nc.vector.reciprocal(rv[:], var[:])
            rstd = sbp.tile([P, 1], F32, tag="rstd", name=f"rstd{mt}")
            nc.scalar.activation(rstd[:], rv[:], AF.Sqrt)
            yn = sbp.tile([P, E], F32, tag="yn", name=f"yn{mt}")
            nc.vector.tensor_scalar(yn[:], x[:], nm[:], rstd[:], op0=ALU.add, op1=ALU.mult)
            yg = sbp.tile([P, E], F32, tag="yg", name=f"yg{mt}")
            nc.vector.tensor_tensor(yg[:], yn[:], gam_bc[:], op=ALU.mult)
            yb = sbp.tile([P, E], F32, tag="yb", name=f"yb{mt}")
            nc.vector.tensor_tensor(yb[:], yg[:], bet_bc[:], op=ALU.add)
            nc.sync.dma_start(y_d.ap().rearrange("(m p) e -> p m e", p=P)[:, mt, :], yb[:])

    nc.compile()
    return nc


def _get(name):
    if name not in _CACHE:
        _CACHE[name] = _build_phase_a() if name == "a" else _build_phase_b()
    return _CACHE[name]


def kernel(query, key_value, key_value_mask, Wq, bq, Wk, bk, Wv, bv, Wo, bo,
           ln_gamma, ln_beta):
    f = lambda a: np.ascontiguousarray(np.asarray(a, dtype=np.float32))
    query, key_value = f(query), f(key_value)
    Wq, Wk, Wv, Wo = f(Wq), f(Wk), f(Wv), f(Wo)
    bq, bk, bv, bo = f(bq), f(bk), f(bv), f(bo)
    ln_gamma, ln_beta = f(ln_gamma), f(ln_beta)
    mask01 = (np.asarray(key_value_mask) != 0).astype(np.float32)

    def shuf(w):
        # [p, j*256+d] = W.T[j*128+p, d] -> contiguous 8KB DMA rows
        return f(w.T.reshape(NE, P, HD).transpose(1, 0, 2).reshape(P, NE * HD))

    nc_a = _get("a")
    in_maps_a = []
    for c in range(8):
        b, hg = c // 4, c % 4
        sl = slice(hg * HD, (hg + 1) * HD)
        in_maps_a.append({
            "qT": f(query[b].T),
            "kvT": f(key_value[b].T),
            "wqT": shuf(Wq[sl]),
            "wkT": shuf(Wk[sl]),
            "wvT": shuf(Wv[sl]),
            "bq": bq[sl].reshape(1, HD),
            "bk": bk[sl].reshape(1, HD),
            "bv": bv[sl].reshape(1, HD),
            "mask01": f(mask01[b].reshape(NKV, P).T),
        })
    res_a = run_bass_kernel_spmd(nc_a, in_maps_a, core_ids=list(range(8)))
    attnT = [np.concatenate([res_a.results[b * 4 + hg]["attnT"] for hg in range(4)], axis=0)
             for b in range(B)]   # per batch: (1024 dims, 1024 q)

    nc_b = _get("b")
    woT = f(Wo.T)
    bo_r = bo.reshape(1, E)
    gam_r = ln_gamma.reshape(1, E)
    bet_r = ln_beta.reshape(1, E)
    in_maps_b = []
    for c in range(8):
        b, j = c // 4, c % 4
        rs = slice(j * 256, (j + 1) * 256)
        in_maps_b.append({
            "aT": f(attnT[b][:, rs]),
            "woT": woT,
            "qn": f(query[b, rs, :]),
            "bo": bo_r,
            "gam": gam_r,
            "bet": bet_r,
        })
    res_b = run_bass_kernel_spmd(nc_b, in_maps_b, core_ids=list(range(8)))
    out = np.empty((B, SQ, E), np.float32)
    for c in range(8):
        b, j = c // 4, c % 4
        out[b, j * 256:(j + 1) * 256, :] = res_b.results[c]["y"]
    return out



# revision 7
# speedup vs baseline: 1.5676x; 1.5676x over previous
"""CrossAttentionBlock on 8 trn2 NeuronCores.

Sharding: data parallel over batch B=2, tensor parallel over heads
(16 heads -> 4 groups of 4). Core c = b*4 + hg.

Key ideas vs the straightforward version:
  - kv compaction: the mask zeroes ~50% of kv positions, and masked softmax
    over the full sequence is EXACTLY softmax over the unmasked subset. The
    host gathers unmasked kv rows (pad to a multiple of 512), halving K/V
    projections, scores, exp and attn@V on device. Padded rows carry a 0
    mask column so they drop out of the denominator; their V rows are 0.
  - fp8 (e4m3) operands everywhere on the PE: inputs/weights are cast on
    the host (pure dtype marshaling), intermediates (q/k/v/exp) are cast
    for free during PSUM evacuation. Projections and attn@V run in
    DoubleRow perf mode (2 contraction rows/cycle).
  - attn@V flipped: out[q(128), d] instead of out[d, q(512)] -> 65-row
    moving operand per call at 0.5 cycles/row.
  - exp computed as exp(s/8 - 2): keeps values <= ~30, inside e4m3 range;
    numerator and denominator scale identically so the ratio is unchanged.
  - softmax denominator from a constant 1.0 column appended to V (masked),
    so no row-max / no bias pass is needed (scores are N(0,1)-scale).

Phase A output: normalized attention [SQ, 256] bf16 per core.
Phase B: rows sharded (256 rows of B*SQ each): out = attn @ Wo.T + bo +
residual, then LayerNorm.
"""

import math
import numpy as np
from contextlib import ExitStack

import ml_dtypes

import concourse.bacc as bacc
import concourse.tile as tile
import concourse.mybir as mybir
from concourse.bass_utils import run_bass_kernel_spmd

F32 = mybir.dt.float32
F32R = mybir.dt.float32r
BF16 = mybir.dt.bfloat16
FP8 = mybir.dt.float8e4
NP_FP8 = ml_dtypes.float8_e4m3
NP_BF16 = ml_dtypes.bfloat16
AF = mybir.ActivationFunctionType
ALU = mybir.AluOpType
AX = mybir.AxisListType
DR = mybir.MatmulPerfMode.DoubleRow

B, SQ, SKV, E = 2, 1024, 4096, 1024
H, D = 16, 64
HG = 4                 # heads per core
HD = HG * D            # 256
P = 128
NE = E // P            # 8
LN_EPS = 1e-5
SCALE = 1.0 / np.sqrt(D)
EXP_SHIFT = -2.0       # exp(s*SCALE + EXP_SHIFT): keeps e4m3 in range

_CACHE = {}


def _build_phase_a(KVP):
    NKV = KVP // P
    nc = bacc.Bacc("TRN2", target_bir_lowering=False, debug=False, num_devices=8)

    qT_d = nc.dram_tensor("qT", [E, SQ], FP8, kind="ExternalInput")
    kvT_d = nc.dram_tensor("kvT", [E, KVP], FP8, kind="ExternalInput")
    wqT_d = nc.dram_tensor("wqT", [P, NE * HD], FP8, kind="ExternalInput")
    wkT_d = nc.dram_tensor("wkT", [P, NE * HD], FP8, kind="ExternalInput")
    wvT_d = nc.dram_tensor("wvT", [P, NE * HD], FP8, kind="ExternalInput")
    bqT_d = nc.dram_tensor("bqT", [P, 2], F32, kind="ExternalInput")
    bkT_d = nc.dram_tensor("bkT", [P, 2], F32, kind="ExternalInput")
    bv_d = nc.dram_tensor("bv", [1, HD], F32, kind="ExternalInput")
    mask_d = nc.dram_tensor("mask01", [P, NKV], F32, kind="ExternalInput")
    attn_d = nc.dram_tensor("attn", [SQ, HD], BF16, kind="ExternalOutput")

    with tile.TileContext(nc) as tc, ExitStack() as ctx:
        const = ctx.enter_context(tc.tile_pool(name="const", bufs=1))

        wq_sb = const.tile([P, NE, HD], FP8)
        nc.sync.dma_start(wq_sb[:], wqT_d.ap().rearrange("p (j d) -> p j d", d=HD))
        wk_sb = const.tile([P, NE, HD], FP8)
        nc.sync.dma_start(wk_sb[:], wkT_d.ap().rearrange("p (j d) -> p j d", d=HD))
        wv_sb = const.tile([P, NE, HD], FP8)
        nc.sync.dma_start(wv_sb[:], wvT_d.ap().rearrange("p (j d) -> p j d", d=HD))
        bq_sb = const.tile([P, 2], F32)
        nc.sync.dma_start(bq_sb[:], bqT_d.ap())
        bk_sb = const.tile([P, 2], F32)
        nc.sync.dma_start(bk_sb[:], bkT_d.ap())
        bv_sb = const.tile([1, HD], F32)
        nc.sync.dma_start(bv_sb[:], bv_d.ap())
        mask_sb = const.tile([P, NKV], F32)
        nc.sync.dma_start(mask_sb[:], mask_d.ap())

        shift_sb = const.tile([P, 1], F32)
        nc.any.memset(shift_sb[:], 0.0)
        nc.vector.tensor_scalar(shift_sb[:], shift_sb[:], EXP_SHIFT, None, op0=ALU.add)

        # bv broadcast to all partitions (K=1 matmul); bv is usually zero but
        # the add is cheap and keeps the kernel general.
        ones32_sb = const.tile([1, P], F32)
        nc.any.memset(ones32_sb[:], 1.0)
        ones_sb = const.tile([1, P], F32R)
        nc.vector.tensor_copy(ones_sb[:], ones32_sb[:])
        bvr_sb = const.tile([1, HD], F32R)
        nc.vector.tensor_copy(bvr_sb[:], bv_sb[:])
        bv_bc = const.tile([P, HD], F32)

        qTs_sb = const.tile([P, 2, SQ], FP8)          # q^T, dims on partitions
        kT_sb = const.tile([P, 2, KVP], FP8)          # k^T, dims on partitions
        v_sb = const.tile([P, NKV, HG * (D + 1)], FP8)  # v rows + mask column
        attn_sb = const.tile([P, SQ // P, HD], BF16)  # output, q on partitions

        # mask columns: v[:, t, 65*h + 64] = mask tile t
        for h in range(HG):
            nc.vector.tensor_copy(
                v_sb[:].rearrange("p t (h u) -> p t h u", u=D + 1)[:, :, h, D],
                mask_sb[:])

        with ExitStack() as r0:
            ck_pool = r0.enter_context(tc.tile_pool(name="ck", bufs=3))
            pj_ps = r0.enter_context(tc.tile_pool(name="pjps", bufs=2, space="PSUM"))
            gb_ps = r0.enter_context(tc.tile_pool(name="gbps", bufs=1, space="PSUM"))

            psb = gb_ps.tile([P, HD], F32, tag="gb", name="bvbc")
            nc.tensor.matmul(psb[:], ones_sb[:], bvr_sb[:], start=True, stop=True)
            nc.scalar.copy(bv_bc[:], psb[:])

            def dr_proj(ps, w_sb, ch, m):
                for j2 in range(NE // 2):
                    nc.tensor.matmul(ps[:], w_sb[:, 2 * j2:2 * j2 + 2, m * P:(m + 1) * P],
                                     ch[:, 2 * j2:2 * j2 + 2, :],
                                     start=(j2 == 0), stop=(j2 == NE // 2 - 1),
                                     perf_mode=DR)

            for c in range(SQ // 512):
                ch = ck_pool.tile([P, NE, 512], FP8, tag="ch", name=f"chq{c}")
                for j in range(NE):
                    nc.sync.dma_start(ch[:, j, :], qT_d.ap()[j * P:(j + 1) * P, c * 512:(c + 1) * 512])
                for m in range(2):
                    ps = pj_ps.tile([P, 512], F32, tag="pj", name=f"qps{c}_{m}")
                    dr_proj(ps, wq_sb, ch, m)
                    nc.vector.tensor_scalar(qTs_sb[:, m, c * 512:(c + 1) * 512],
                                            ps[:], bq_sb[:, m:m + 1], None, op0=ALU.add)

            for c in range(KVP // 512):
                ch = ck_pool.tile([P, NE, 512], FP8, tag="ch", name=f"chkv{c}")
                for j in range(NE):
                    nc.sync.dma_start(ch[:, j, :], kvT_d.ap()[j * P:(j + 1) * P, c * 512:(c + 1) * 512])
                for m in range(2):
                    ps = pj_ps.tile([P, 512], F32, tag="pj", name=f"kps{c}_{m}")
                    dr_proj(ps, wk_sb, ch, m)
                    nc.vector.tensor_scalar(kT_sb[:, m, c * 512:(c + 1) * 512],
                                            ps[:], bk_sb[:, m:m + 1], None, op0=ALU.add)
                for tt in range(4):
                    t = 4 * c + tt
                    ps = pj_ps.tile([P, HD], F32, tag="pj", name=f"vps{t}")
                    for j2 in range(NE // 2):
                        nc.tensor.matmul(ps[:], ch[:, 2 * j2:2 * j2 + 2, tt * P:(tt + 1) * P],
                                         wv_sb[:, 2 * j2:2 * j2 + 2, :],
                                         start=(j2 == 0), stop=(j2 == NE // 2 - 1),
                                         perf_mode=DR)
                    nc.vector.tensor_copy(
                        v_sb[:, t, :].rearrange("p (h u) -> p h u", u=D + 1)[:, :, 0:D],
                        ps[:].rearrange("p (h d) -> p h d", d=D))

        # ---- attention: per q-half, accumulate all 4 heads x 4 q-subtiles
        with ExitStack() as r1:
            sc_ps = r1.enter_context(tc.tile_pool(name="scps", bufs=2, space="PSUM"))
            pv_ps = r1.enter_context(tc.tile_pool(name="pvps", bufs=4, space="PSUM"))
            ex_pool = r1.enter_context(tc.tile_pool(name="expool", bufs=8))
            sm_pool = r1.enter_context(tc.tile_pool(name="smpool", bufs=4))

            NPAIR = NKV // 2
            for qh in range(2):
                pv = [pv_ps.tile([P, HG, D + 1], F32, tag="pv", name=f"pv{qh}_{qq}")
                      for qq in range(4)]
                for i in range(NPAIR):
                    for h in range(HG):
                        m, doff = h // 2, (h % 2) * 64
                        sc = sc_ps.tile([P, 2, 512], F32, tag="sc", name=f"s{qh}_{i}_{h}")
                        for tt in range(2):
                            t = 2 * i + tt
                            nc.tensor.matmul(
                                sc[:, tt, :],
                                kT_sb[doff:doff + D, m, t * P:(t + 1) * P],
                                qTs_sb[doff:doff + D, m, qh * 512:(qh + 1) * 512],
                                start=True, stop=True)
                        ex = ex_pool.tile([P, 2, 512], FP8, tag="ex", name=f"e{qh}_{i}_{h}")
                        nc.scalar.activation(ex[:], sc[:], AF.Exp,
                                             bias=shift_sb[:], scale=float(SCALE))
                        for qq in range(4):
                            nc.tensor.matmul(
                                pv[qq][:, h, :],
                                ex[:, 0:2, qq * P:(qq + 1) * P],
                                v_sb[:, 2 * i:2 * i + 2, h * (D + 1):(h + 1) * (D + 1)],
                                start=(i == 0), stop=(i == NPAIR - 1),
                                perf_mode=DR)
                for qq in range(4):
                    g = qh * 4 + qq
                    for h in range(HG):
                        rec = sm_pool.tile([P, 1], F32, tag="rec", name=f"rec{g}_{h}")
                        nc.vector.reciprocal(rec[:], pv[qq][:, h, D:D + 1])
                        nc.vector.tensor_scalar(
                            attn_sb[:, g, h * D:(h + 1) * D],
                            pv[qq][:, h, 0:D], rec[:], None, op0=ALU.mult)
                    nc.vector.tensor_tensor(attn_sb[:, g, :], attn_sb[:, g, :],
                                            bv_bc[:], op=ALU.add)

        nc.sync.dma_start(attn_d.ap().rearrange("(g p) d -> p g d", p=P), attn_sb[:])

    nc.compile()
    return nc


def _build_phase_b():
    R = 2 * P   # 256 rows per core
    nc = bacc.Bacc("TRN2", target_bir_lowering=False, debug=False, num_devices=8)

    aT_d = nc.dram_tensor("aT", [E, R], BF16, kind="ExternalInput")
    woT_d = nc.dram_tensor("woT", [E, E], BF16, kind="ExternalInput")
    qn_d = nc.dram_tensor("qn", [R, E], F32, kind="ExternalInput")
    bo_d = nc.dram_tensor("bo", [1, E], F32R, kind="ExternalInput")
    gam_d = nc.dram_tensor("gam", [1, E], F32R, kind="ExternalInput")
    bet_d = nc.dram_tensor("bet", [1, E], F32R, kind="ExternalInput")
    y_d = nc.dram_tensor("y", [R, E], F32, kind="ExternalOutput")

    with tile.TileContext(nc) as tc, ExitStack() as ctx:
        const = ctx.enter_context(tc.tile_pool(name="const", bufs=1))
        aT_sb = const.tile([P, NE, R], BF16)
        for k in range(NE):
            nc.sync.dma_start(aT_sb[:, k, :], aT_d.ap()[k * P:(k + 1) * P, :])
        qn_sb = const.tile([P, 2, E], F32)
        for mt in range(2):
            nc.sync.dma_start(qn_sb[:, mt, :], qn_d.ap().rearrange("(m p) e -> p m e", p=P)[:, mt, :])
        bo_sb = const.tile([1, E], F32R)
        nc.sync.dma_start(bo_sb[:], bo_d.ap())
        gam_sb = const.tile([1, E], F32R)
        nc.sync.dma_start(gam_sb[:], gam_d.ap())
        bet_sb = const.tile([1, E], F32R)
        nc.sync.dma_start(bet_sb[:], bet_d.ap())
        ones32_sb = const.tile([1, P], F32)
        nc.any.memset(ones32_sb[:], 1.0)
        ones_sb = const.tile([1, P], F32R)
        nc.vector.tensor_copy(ones_sb[:], ones32_sb[:])

        gam_bc = const.tile([P, E], F32)
        bet_bc = const.tile([P, E], F32)

        wo_pool = ctx.enter_context(tc.tile_pool(name="wo", bufs=4))
        ps_pool = ctx.enter_context(tc.tile_pool(name="ps", bufs=2, space="PSUM"))
        gb_ps = ctx.enter_context(tc.tile_pool(name="gbps", bufs=2, space="PSUM"))
        sbp = ctx.enter_context(tc.tile_pool(name="sbp", bufs=2))

        ps_tiles = {}
        for k in range(NE):
            wo = wo_pool.tile([P, E], BF16, tag="wo", name=f"wo{k}")
            nc.sync.dma_start(wo[:], woT_d.ap()[k * P:(k + 1) * P, :])
            for mt in range(2):
                if k == 0:
                    ps_tiles[mt] = ps_pool.tile([P, E], F32, tag="o", name=f"o{mt}")
                for nh in range(2):
                    nc.tensor.matmul(ps_tiles[mt][:, nh * 512:(nh + 1) * 512],
                                     aT_sb[:, k, mt * P:(mt + 1) * P],
                                     wo[:, nh * 512:(nh + 1) * 512],
                                     start=(k == 0), stop=False)
        for mt in range(2):
            for nh in range(2):
                nc.tensor.matmul(ps_tiles[mt][:, nh * 512:(nh + 1) * 512], ones_sb[:],
                                 bo_sb[:, nh * 512:(nh + 1) * 512],
                                 start=False, stop=True)

        # broadcast gamma/beta rows to all 128 partitions via K=1 matmuls
        for half in range(2):
            cs = slice(half * 512, (half + 1) * 512)
            psg = gb_ps.tile([P, 512], F32, tag="gb", name=f"gbg{half}")
            nc.tensor.matmul(psg[:], ones_sb[:], gam_sb[:, cs], start=True, stop=True)
            nc.scalar.copy(gam_bc[:, cs], psg[:])
            psb = gb_ps.tile([P, 512], F32, tag="gb", name=f"gbb{half}")
            nc.tensor.matmul(psb[:], ones_sb[:], bet_sb[:, cs], start=True, stop=True)
            nc.scalar.copy(bet_bc[:, cs], psb[:])

        # residual + LayerNorm, var = E[x^2] - mean^2 so the two reductions
        # run on different engines (DVE reduce, ACT Square+accum) in parallel
        for mt in range(2):
            x = sbp.tile([P, E], F32, tag="x", name=f"x{mt}")
            nc.vector.tensor_tensor(x[:], ps_tiles[mt][:], qn_sb[:, mt, :], op=ALU.add)
            s1 = sbp.tile([P, 1], F32, tag="s1", name=f"s1{mt}")
            nc.vector.reduce_sum(s1[:], x[:], axis=AX.X)
            sq = sbp.tile([P, E], F32, tag="sq", name=f"sq{mt}")
            ssq = sbp.tile([P, 1], F32, tag="ssq", name=f"ssq{mt}")
            nc.scalar.activation(sq[:], x[:], AF.Square, accum_out=ssq[:])
            nm = sbp.tile([P, 1], F32, tag="nm", name=f"nm{mt}")
            nc.vector.tensor_scalar(nm[:], s1[:], -1.0 / E, None, op0=ALU.mult)
            m2 = sbp.tile([P, 1], F32, tag="m2", name=f"m2{mt}")
            nc.vector.tensor_tensor(m2[:], nm[:], nm[:], op=ALU.mult)
            var = sbp.tile([P, 1], F32, tag="var", name=f"var{mt}")
            nc.vector.tensor_scalar(var[:], ssq[:], 1.0 / E, LN_EPS, op0=ALU.mult, op1=ALU.add)
            nc.vector.tensor_tensor(var[:], var[:], m2[:], op=ALU.subtract)
            rv = sbp.tile([P, 1], F32, tag="rv", name=f"rv{mt}")
            nc.vector.reciprocal(rv[:], var[:])
            rstd = sbp.tile([P, 1], F32, tag="rstd", name=f"rstd{mt}")
            nc.scalar.activation(rstd[:], rv[:], AF.Sqrt)
            yn = sbp.tile([P, E], F32, tag="yn", name=f"yn{mt}")
            nc.vector.tensor_scalar(yn[:], x[:], nm[:], rstd[:], op0=ALU.add, op1=ALU.mult)
            yg = sbp.tile([P, E], F32, tag="yg", name=f"yg{mt}")
            nc.vector.tensor_tensor(yg[:], yn[:], gam_bc[:], op=ALU.mult)
            yb = sbp.tile([P, E], F32, tag="yb", name=f"yb{mt}")
            nc.vector.tensor_tensor(yb[:], yg[:], bet_bc[:], op=ALU.add)
            nc.sync.dma_start(y_d.ap().rearrange("(m p) e -> p m e", p=P)[:, mt, :], yb[:])

    nc.compile()
    return nc


def _get(name, KVP=None):
    key = (name, KVP)
    if key not in _CACHE:
        _CACHE[key] = _build_phase_a(KVP) if name == "a" else _build_phase_b()
    return _CACHE[key]


def kernel(query, key_value, key_value_mask, Wq, bq, Wk, bk, Wv, bv, Wo, bo,
           ln_gamma, ln_beta):
    f = lambda a: np.ascontiguousarray(np.asarray(a, dtype=np.float32))
    f8 = lambda a: np.ascontiguousarray(np.asarray(a).astype(NP_FP8))
    fb = lambda a: np.ascontiguousarray(np.asarray(a).astype(NP_BF16))
    query, key_value = f(query), f(key_value)
    Wq, Wk, Wv, Wo = f(Wq), f(Wk), f(Wv), f(Wo)
    bq, bk, bv, bo = f(bq), f(bk), f(bv), f(bo)
    ln_gamma, ln_beta = f(ln_gamma), f(ln_beta)
    maskb = np.asarray(key_value_mask) != 0

    # compact kv: softmax over masked scores == softmax over unmasked subset
    cnt = [int(maskb[b].sum()) for b in range(B)]
    KVP = max(512, int(512 * math.ceil(max(cnt) / 512)))
    NKV = KVP // P
    kvc = np.zeros((B, KVP, E), np.float32)
    mask01 = np.zeros((B, KVP), np.float32)
    for b in range(B):
        kvc[b, :cnt[b]] = key_value[b][maskb[b]]
        mask01[b, :cnt[b]] = 1.0

    def shuf(w):
        # [p, j*256+d] = W.T[j*128+p, d] -> contiguous 1KB DMA rows
        return np.ascontiguousarray(
            w.T.reshape(NE, P, HD).transpose(1, 0, 2).reshape(P, NE * HD)).astype(NP_FP8)

    nc_a = _get("a", KVP)
    in_maps_a = []
    for c in range(8):
        b, hg = c // 4, c % 4
        sl = slice(hg * HD, (hg + 1) * HD)
        in_maps_a.append({
            "qT": f8(query[b].T),
            "kvT": f8(kvc[b].T),
            "wqT": shuf(Wq[sl]),
            "wkT": shuf(Wk[sl]),
            "wvT": shuf(Wv[sl]),
            "bqT": f(bq[sl].reshape(2, P).T),
            "bkT": f(bk[sl].reshape(2, P).T),
            "bv": bv[sl].reshape(1, HD),
            "mask01": f(mask01[b].reshape(NKV, P).T),
        })
    res_a = run_bass_kernel_spmd(nc_a, in_maps_a, core_ids=list(range(8)))
    # per batch: [SQ, E] bf16 attention output (concat head groups on dims)
    attn = [np.concatenate([res_a.results[b * 4 + hg]["attn"] for hg in range(4)], axis=1)
            for b in range(B)]

    nc_b = _get("b")
    woT = fb(Wo.T)
    bo_r = bo.reshape(1, E)
    gam_r = ln_gamma.reshape(1, E)
    bet_r = ln_beta.reshape(1, E)
    in_maps_b = []
    for c in range(8):
        b, j = c // 4, c % 4
        rs = slice(j * 256, (j + 1) * 256)
        in_maps_b.append({
            "aT": np.ascontiguousarray(attn[b].T[:, rs]),
            "woT": woT,
            "qn": f(query[b, rs, :]),
            "bo": bo_r,
            "gam": gam_r,
            "bet": bet_r,
        })
    res_b = run_bass_kernel_spmd(nc_b, in_maps_b, core_ids=list(range(8)))
    out = np.empty((B, SQ, E), np.float32)
    for c in range(8):
        b, j = c // 4, c % 4
        out[b, j * 256:(j + 1) * 256, :] = res_b.results[c]["y"]
    return out


# revision 12
# speedup vs baseline: 1.6484x; 1.0515x over previous
"""CrossAttentionBlock on 8 trn2 NeuronCores.

Sharding: data parallel over batch B=2, tensor parallel over heads
(16 heads -> 4 groups of 4). Core c = b*4 + hg.

Key ideas vs the straightforward version:
  - kv compaction: the mask zeroes ~50% of kv positions, and masked softmax
    over the full sequence is EXACTLY softmax over the unmasked subset. The
    host gathers unmasked kv rows (pad to a multiple of 512), halving K/V
    projections, scores, exp and attn@V on device. Padded rows carry a 0
    mask column so they drop out of the denominator; their V rows are 0.
  - fp8 (e4m3) operands everywhere on the PE: inputs/weights are cast on
    the host (pure dtype marshaling), intermediates (q/k/v/exp) are cast
    for free during PSUM evacuation. Projections and attn@V run in
    DoubleRow perf mode (2 contraction rows/cycle).
  - attn@V flipped: out[q(128), d] instead of out[d, q(512)] -> 65-row
    moving operand per call at 0.5 cycles/row.
  - exp computed as exp(s/8 - 2): keeps values <= ~30, inside e4m3 range;
    numerator and denominator scale identically so the ratio is unchanged.
  - softmax denominator from a constant 1.0 column appended to V (masked),
    so no row-max / no bias pass is needed (scores are N(0,1)-scale).

Phase A output: normalized attention [SQ, 256] bf16 per core.
Phase B: rows sharded (256 rows of B*SQ each): out = attn @ Wo.T + bo +
residual, then LayerNorm.
"""

import math
import numpy as np
from contextlib import ExitStack

import ml_dtypes

import concourse.bacc as bacc
import concourse.tile as tile
import concourse.mybir as mybir
from concourse.bass_utils import run_bass_kernel_spmd

F32 = mybir.dt.float32
F32R = mybir.dt.float32r
BF16 = mybir.dt.bfloat16
FP8 = mybir.dt.float8e4
NP_FP8 = ml_dtypes.float8_e4m3
NP_BF16 = ml_dtypes.bfloat16
AF = mybir.ActivationFunctionType
ALU = mybir.AluOpType
AX = mybir.AxisListType
DR = mybir.MatmulPerfMode.DoubleRow

B, SQ, SKV, E = 2, 1024, 4096, 1024
H, D = 16, 64
HG = 4                 # heads per core
HD = HG * D            # 256
P = 128
NE = E // P            # 8
LN_EPS = 1e-5
SCALE = 1.0 / np.sqrt(D)
EXP_SHIFT = -2.0       # exp(s*SCALE + EXP_SHIFT): keeps e4m3 in range

_CACHE = {}


def _build_phase_a(KVP):
    NKV = KVP // P
    nc = bacc.Bacc("TRN2", target_bir_lowering=False, debug=False, num_devices=8)

    qT_d = nc.dram_tensor("qT", [E, SQ], FP8, kind="ExternalInput")
    kvT_d = nc.dram_tensor("kvT", [E, KVP], FP8, kind="ExternalInput")
    wqT_d = nc.dram_tensor("wqT", [P, NE * HD], FP8, kind="ExternalInput")
    wkT_d = nc.dram_tensor("wkT", [P, NE * HD], FP8, kind="ExternalInput")
    wvT_d = nc.dram_tensor("wvT", [P, NE * HD], FP8, kind="ExternalInput")
    bqT_d = nc.dram_tensor("bqT", [P, 2], F32, kind="ExternalInput")
    bkT_d = nc.dram_tensor("bkT", [P, 2], F32, kind="ExternalInput")
    bv_d = nc.dram_tensor("bv", [1, HD], F32, kind="ExternalInput")
    mask_d = nc.dram_tensor("mask01", [P, NKV], F32, kind="ExternalInput")
    attn_d = nc.dram_tensor("attn", [SQ, HD], BF16, kind="ExternalOutput")

    with tile.TileContext(nc) as tc, ExitStack() as ctx:
        const = ctx.enter_context(tc.tile_pool(name="const", bufs=1))

        wq_sb = const.tile([P, NE, HD], FP8)
        nc.sync.dma_start(wq_sb[:], wqT_d.ap().rearrange("p (j d) -> p j d", d=HD))
        wk_sb = const.tile([P, NE, HD], FP8)
        nc.sync.dma_start(wk_sb[:], wkT_d.ap().rearrange("p (j d) -> p j d", d=HD))
        wv_sb = const.tile([P, NE, HD], FP8)
        nc.sync.dma_start(wv_sb[:], wvT_d.ap().rearrange("p (j d) -> p j d", d=HD))
        bq_sb = const.tile([P, 2], F32)
        nc.sync.dma_start(bq_sb[:], bqT_d.ap())
        bk_sb = const.tile([P, 2], F32)
        nc.sync.dma_start(bk_sb[:], bkT_d.ap())
        bv_sb = const.tile([1, HD], F32)
        nc.sync.dma_start(bv_sb[:], bv_d.ap())
        mask_sb = const.tile([P, NKV], F32)
        nc.sync.dma_start(mask_sb[:], mask_d.ap())

        shift_sb = const.tile([P, 1], F32)
        nc.any.memset(shift_sb[:], 0.0)
        nc.vector.tensor_scalar(shift_sb[:], shift_sb[:], EXP_SHIFT, None, op0=ALU.add)

        # bv broadcast to all partitions (K=1 matmul); bv is usually zero but
        # the add is cheap and keeps the kernel general.
        ones32_sb = const.tile([1, P], F32)
        nc.any.memset(ones32_sb[:], 1.0)
        ones_sb = const.tile([1, P], F32R)
        nc.vector.tensor_copy(ones_sb[:], ones32_sb[:])
        bvr_sb = const.tile([1, HD], F32R)
        nc.vector.tensor_copy(bvr_sb[:], bv_sb[:])
        bv_bc = const.tile([P, HD], F32)

        qTs_sb = const.tile([P, 2, SQ], FP8)          # q^T, dims on partitions
        kT_sb = const.tile([P, 2, KVP], FP8)          # k^T, dims on partitions
        v_sb = const.tile([P, NKV, HG * (D + 1)], FP8)  # v rows + mask column
        attn_sb = const.tile([P, SQ // P, HD], BF16)  # output, q on partitions

        # mask columns: v[:, t, 65*h + 64] = mask tile t
        for h in range(HG):
            nc.vector.tensor_copy(
                v_sb[:].rearrange("p t (h u) -> p t h u", u=D + 1)[:, :, h, D],
                mask_sb[:])

        with ExitStack() as r0:
            ck_pool = r0.enter_context(tc.tile_pool(name="ck", bufs=3))
            pj_ps = r0.enter_context(tc.tile_pool(name="pjps", bufs=2, space="PSUM"))
            gb_ps = r0.enter_context(tc.tile_pool(name="gbps", bufs=1, space="PSUM"))

            psb = gb_ps.tile([P, HD], F32, tag="gb", name="bvbc")
            nc.tensor.matmul(psb[:], ones_sb[:], bvr_sb[:], start=True, stop=True)
            nc.scalar.copy(bv_bc[:], psb[:])

            def dr_proj(ps, w_sb, ch, m):
                for j2 in range(NE // 2):
                    nc.tensor.matmul(ps[:], w_sb[:, 2 * j2:2 * j2 + 2, m * P:(m + 1) * P],
                                     ch[:, 2 * j2:2 * j2 + 2, :],
                                     start=(j2 == 0), stop=(j2 == NE // 2 - 1),
                                     perf_mode=DR)

            for c in range(SQ // 512):
                ch = ck_pool.tile([P, NE, 512], FP8, tag="ch", name=f"chq{c}")
                for j in range(NE):
                    nc.sync.dma_start(ch[:, j, :], qT_d.ap()[j * P:(j + 1) * P, c * 512:(c + 1) * 512])
                for m in range(2):
                    ps = pj_ps.tile([P, 512], F32, tag="pj", name=f"qps{c}_{m}")
                    dr_proj(ps, wq_sb, ch, m)
                    nc.vector.tensor_scalar(qTs_sb[:, m, c * 512:(c + 1) * 512],
                                            ps[:], bq_sb[:, m:m + 1], None, op0=ALU.add)

            for c in range((KVP + 511) // 512):
                w = min(512, KVP - c * 512)
                ch = ck_pool.tile([P, NE, w], FP8, tag="ch", name=f"chkv{c}")
                for j in range(NE):
                    nc.sync.dma_start(ch[:, j, :], kvT_d.ap()[j * P:(j + 1) * P, c * 512:c * 512 + w])
                for m in range(2):
                    ps = pj_ps.tile([P, w], F32, tag="pj", name=f"kps{c}_{m}")
                    dr_proj(ps, wk_sb, ch, m)
                    nc.vector.tensor_scalar(kT_sb[:, m, c * 512:c * 512 + w],
                                            ps[:], bk_sb[:, m:m + 1], None, op0=ALU.add)
                for tt in range(w // P):
                    t = 4 * c + tt
                    ps = pj_ps.tile([P, HD], F32, tag="pj", name=f"vps{t}")
                    for j2 in range(NE // 2):
                        nc.tensor.matmul(ps[:], ch[:, 2 * j2:2 * j2 + 2, tt * P:(tt + 1) * P],
                                         wv_sb[:, 2 * j2:2 * j2 + 2, :],
                                         start=(j2 == 0), stop=(j2 == NE // 2 - 1),
                                         perf_mode=DR)
                    nc.vector.tensor_copy(
                        v_sb[:, t, :].rearrange("p (h u) -> p h u", u=D + 1)[:, :, 0:D],
                        ps[:].rearrange("p (h d) -> p h d", d=D))

        # ---- attention: per q-half, accumulate all 4 heads x 4 q-subtiles
        with ExitStack() as r1:
            sc_ps = r1.enter_context(tc.tile_pool(name="scps", bufs=2, space="PSUM"))
            pv_ps = r1.enter_context(tc.tile_pool(name="pvps", bufs=4, space="PSUM"))
            ex_pool = r1.enter_context(tc.tile_pool(name="expool", bufs=8))
            sm_pool = r1.enter_context(tc.tile_pool(name="smpool", bufs=4))

            NPAIR = NKV // 2

            def att_av(qh, pv, i, h, ex):
                for qq in range(4):
                    nc.tensor.matmul(
                        pv[qq][:, h, :],
                        ex[:, 0:2, qq * P:(qq + 1) * P],
                        v_sb[:, 2 * i:2 * i + 2, h * (D + 1):(h + 1) * (D + 1)],
                        start=(i == 0), stop=(i == NPAIR - 1),
                        perf_mode=DR)

            for qh in range(2):
                pv = [pv_ps.tile([P, HG, D + 1], F32, tag="pv", name=f"pv{qh}_{qq}")
                      for qq in range(4)]
                pend = None
                for i in range(NPAIR):
                    for h in range(HG):
                        m, doff = h // 2, (h % 2) * 64
                        sc = sc_ps.tile([P, 2, 512], F32, tag="sc", name=f"s{qh}_{i}_{h}")
                        for tt in range(2):
                            t = 2 * i + tt
                            nc.tensor.matmul(
                                sc[:, tt, :],
                                kT_sb[doff:doff + D, m, t * P:(t + 1) * P],
                                qTs_sb[doff:doff + D, m, qh * 512:(qh + 1) * 512],
                                start=True, stop=True)
                        ex = ex_pool.tile([P, 2, 512], FP8, tag="ex", name=f"e{qh}_{i}_{h}")
                        nc.scalar.activation(ex[:], sc[:], AF.Exp,
                                             bias=shift_sb[:], scale=float(SCALE))
                        # 1-(pair,head) emission skew: attn@V for the previous
                        # exp tile issues behind the current scores, so the PE
                        # never waits on the exp it just requested.
                        if pend is not None:
                            att_av(qh, pv, *pend)
                        pend = (i, h, ex)
                att_av(qh, pv, *pend)
                for qq in range(4):
                    g = qh * 4 + qq
                    for h in range(HG):
                        rec = sm_pool.tile([P, 1], F32, tag="rec", name=f"rec{g}_{h}")
                        nc.vector.reciprocal(rec[:], pv[qq][:, h, D:D + 1])
                        nc.vector.tensor_scalar(
                            attn_sb[:, g, h * D:(h + 1) * D],
                            pv[qq][:, h, 0:D], rec[:], None, op0=ALU.mult)
                    nc.vector.tensor_tensor(attn_sb[:, g, :], attn_sb[:, g, :],
                                            bv_bc[:], op=ALU.add)

        nc.sync.dma_start(attn_d.ap().rearrange("(g p) d -> p g d", p=P), attn_sb[:])

    nc.compile()
    return nc


def _build_phase_b():
    R = 2 * P   # 256 rows per core
    nc = bacc.Bacc("TRN2", target_bir_lowering=False, debug=False, num_devices=8)

    aT_d = nc.dram_tensor("aT", [E, R], BF16, kind="ExternalInput")
    woT_d = nc.dram_tensor("woT", [E, E], BF16, kind="ExternalInput")
    qn_d = nc.dram_tensor("qn", [R, E], F32, kind="ExternalInput")
    bo_d = nc.dram_tensor("bo", [1, E], F32R, kind="ExternalInput")
    gam_d = nc.dram_tensor("gam", [1, E], F32R, kind="ExternalInput")
    bet_d = nc.dram_tensor("bet", [1, E], F32R, kind="ExternalInput")
    y_d = nc.dram_tensor("y", [R, E], F32, kind="ExternalOutput")

    with tile.TileContext(nc) as tc, ExitStack() as ctx:
        const = ctx.enter_context(tc.tile_pool(name="const", bufs=1))
        aT_sb = const.tile([P, NE, R], BF16)
        for k in range(NE):
            nc.sync.dma_start(aT_sb[:, k, :], aT_d.ap()[k * P:(k + 1) * P, :])
        qn_sb = const.tile([P, 2, E], F32)
        for mt in range(2):
            nc.sync.dma_start(qn_sb[:, mt, :], qn_d.ap().rearrange("(m p) e -> p m e", p=P)[:, mt, :])
        bo_sb = const.tile([1, E], F32R)
        nc.sync.dma_start(bo_sb[:], bo_d.ap())
        gam_sb = const.tile([1, E], F32R)
        nc.sync.dma_start(gam_sb[:], gam_d.ap())
        bet_sb = const.tile([1, E], F32R)
        nc.sync.dma_start(bet_sb[:], bet_d.ap())
        ones32_sb = const.tile([1, P], F32)
        nc.any.memset(ones32_sb[:], 1.0)
        ones_sb = const.tile([1, P], F32R)
        nc.vector.tensor_copy(ones_sb[:], ones32_sb[:])

        gam_bc = const.tile([P, E], F32)
        bet_bc = const.tile([P, E], F32)

        wo_pool = ctx.enter_context(tc.tile_pool(name="wo", bufs=4))
        ps_pool = ctx.enter_context(tc.tile_pool(name="ps", bufs=2, space="PSUM"))
        gb_ps = ctx.enter_context(tc.tile_pool(name="gbps", bufs=2, space="PSUM"))
        sbp = ctx.enter_context(tc.tile_pool(name="sbp", bufs=2))

        # broadcast gamma/beta before the wo loop so the PE queue serves them
        # first and the LN tail never waits on them
        for half in range(2):
            cs = slice(half * 512, (half + 1) * 512)
            psg = gb_ps.tile([P, 512], F32, tag="gb", name=f"gbg{half}")
            nc.tensor.matmul(psg[:], ones_sb[:], gam_sb[:, cs], start=True, stop=True)
            nc.scalar.copy(gam_bc[:, cs], psg[:])
            psb = gb_ps.tile([P, 512], F32, tag="gb", name=f"gbb{half}")
            nc.tensor.matmul(psb[:], ones_sb[:], bet_sb[:, cs], start=True, stop=True)
            nc.scalar.copy(bet_bc[:, cs], psb[:])

        ps_tiles = {}
        for k in range(NE):
            wo = wo_pool.tile([P, E], BF16, tag="wo", name=f"wo{k}")
            nc.sync.dma_start(wo[:], woT_d.ap()[k * P:(k + 1) * P, :])
            for mt in range(2):
                if k == 0:
                    ps_tiles[mt] = ps_pool.tile([P, E], F32, tag="o", name=f"o{mt}")
                for nh in range(2):
                    nc.tensor.matmul(ps_tiles[mt][:, nh * 512:(nh + 1) * 512],
                                     aT_sb[:, k, mt * P:(mt + 1) * P],
                                     wo[:, nh * 512:(nh + 1) * 512],
                                     start=(k == 0), stop=False)
        for mt in range(2):
            for nh in range(2):
                nc.tensor.matmul(ps_tiles[mt][:, nh * 512:(nh + 1) * 512], ones_sb[:],
                                 bo_sb[:, nh * 512:(nh + 1) * 512],
                                 start=False, stop=True)

        # residual + LayerNorm, var = E[x^2] - mean^2 so the two reductions
        # run on different engines (DVE reduce, ACT Square+accum) in parallel
        for mt in range(2):
            x = sbp.tile([P, E], F32, tag="x", name=f"x{mt}")
            nc.vector.tensor_tensor(x[:], ps_tiles[mt][:], qn_sb[:, mt, :], op=ALU.add)
            s1 = sbp.tile([P, 1], F32, tag="s1", name=f"s1{mt}")
            nc.vector.reduce_sum(s1[:], x[:], axis=AX.X)
            sq = sbp.tile([P, E], F32, tag="sq", name=f"sq{mt}")
            ssq = sbp.tile([P, 1], F32, tag="ssq", name=f"ssq{mt}")
            nc.scalar.activation(sq[:], x[:], AF.Square, accum_out=ssq[:])
            nm = sbp.tile([P, 1], F32, tag="nm", name=f"nm{mt}")
            nc.vector.tensor_scalar(nm[:], s1[:], -1.0 / E, None, op0=ALU.mult)
            m2 = sbp.tile([P, 1], F32, tag="m2", name=f"m2{mt}")
            nc.vector.tensor_tensor(m2[:], nm[:], nm[:], op=ALU.mult)
            var = sbp.tile([P, 1], F32, tag="var", name=f"var{mt}")
            nc.vector.tensor_scalar(var[:], ssq[:], 1.0 / E, LN_EPS, op0=ALU.mult, op1=ALU.add)
            nc.vector.tensor_tensor(var[:], var[:], m2[:], op=ALU.subtract)
            rv = sbp.tile([P, 1], F32, tag="rv", name=f"rv{mt}")
            nc.vector.reciprocal(rv[:], var[:])
            rstd = sbp.tile([P, 1], F32, tag="rstd", name=f"rstd{mt}")
            nc.scalar.activation(rstd[:], rv[:], AF.Sqrt)
            yn = sbp.tile([P, E], F32, tag="yn", name=f"yn{mt}")
            nc.vector.tensor_scalar(yn[:], x[:], nm[:], rstd[:], op0=ALU.add, op1=ALU.mult)
            yg = sbp.tile([P, E], F32, tag="yg", name=f"yg{mt}")
            nc.vector.tensor_tensor(yg[:], yn[:], gam_bc[:], op=ALU.mult)
            yb = sbp.tile([P, E], F32, tag="yb", name=f"yb{mt}")
            nc.vector.tensor_tensor(yb[:], yg[:], bet_bc[:], op=ALU.add)
            nc.sync.dma_start(y_d.ap().rearrange("(m p) e -> p m e", p=P)[:, mt, :], yb[:])

    nc.compile()
    return nc


def _get(name, KVP=None):
    key = (name, KVP)
    if key not in _CACHE:
        _CACHE[key] = _build_phase_a(KVP) if name == "a" else _build_phase_b()
    return _CACHE[key]


def kernel(query, key_value, key_value_mask, Wq, bq, Wk, bk, Wv, bv, Wo, bo,
           ln_gamma, ln_beta):
    f = lambda a: np.ascontiguousarray(np.asarray(a, dtype=np.float32))
    f8 = lambda a: np.ascontiguousarray(np.asarray(a).astype(NP_FP8))
    fb = lambda a: np.ascontiguousarray(np.asarray(a).astype(NP_BF16))
    query, key_value = f(query), f(key_value)
    Wq, Wk, Wv, Wo = f(Wq), f(Wk), f(Wv), f(Wo)
    bq, bk, bv, bo = f(bq), f(bk), f(bv), f(bo)
    ln_gamma, ln_beta = f(ln_gamma), f(ln_beta)
    maskb = np.asarray(key_value_mask) != 0

    # compact kv: softmax over masked scores == softmax over unmasked subset
    cnt = [int(maskb[b].sum()) for b in range(B)]
    KVP = max(512, int(256 * math.ceil(max(cnt) / 256)))
    NKV = KVP // P
    kvc = np.zeros((B, KVP, E), np.float32)
    mask01 = np.zeros((B, KVP), np.float32)
    for b in range(B):
        kvc[b, :cnt[b]] = key_value[b][maskb[b]]
        mask01[b, :cnt[b]] = 1.0

    def shuf(w):
        # [p, j*256+d] = W.T[j*128+p, d] -> contiguous 1KB DMA rows
        return np.ascontiguousarray(
            w.T.reshape(NE, P, HD).transpose(1, 0, 2).reshape(P, NE * HD)).astype(NP_FP8)

    nc_a = _get("a", KVP)
    in_maps_a = []
    for c in range(8):
        b, hg = c // 4, c % 4
        sl = slice(hg * HD, (hg + 1) * HD)
        in_maps_a.append({
            "qT": f8(query[b].T),
            "kvT": f8(kvc[b].T),
            "wqT": shuf(Wq[sl]),
            "wkT": shuf(Wk[sl]),
            "wvT": shuf(Wv[sl]),
            "bqT": f(bq[sl].reshape(2, P).T),
            "bkT": f(bk[sl].reshape(2, P).T),
            "bv": bv[sl].reshape(1, HD),
            "mask01": f(mask01[b].reshape(NKV, P).T),
        })
    res_a = run_bass_kernel_spmd(nc_a, in_maps_a, core_ids=list(range(8)))
    # per batch: [SQ, E] bf16 attention output (concat head groups on dims)
    attn = [np.concatenate([res_a.results[b * 4 + hg]["attn"] for hg in range(4)], axis=1)
            for b in range(B)]

    nc_b = _get("b")
    woT = fb(Wo.T)
    bo_r = bo.reshape(1, E)
    gam_r = ln_gamma.reshape(1, E)
    bet_r = ln_beta.reshape(1, E)
    in_maps_b = []
    for c in range(8):
        b, j = c // 4, c % 4
        rs = slice(j * 256, (j + 1) * 256)
        in_maps_b.append({
            "aT": np.ascontiguousarray(attn[b].T[:, rs]),
            "woT": woT,
            "qn": f(query[b, rs, :]),
            "bo": bo_r,
            "gam": gam_r,
            "bet": bet_r,
        })
    res_b = run_bass_kernel_spmd(nc_b, in_maps_b, core_ids=list(range(8)))
    out = np.empty((B, SQ, E), np.float32)
    for c in range(8):
        b, j = c // 4, c % 4
        out[b, j * 256:(j + 1) * 256, :] = res_b.results[c]["y"]
    return out


# revision 27
# speedup vs baseline: 1.8033x; 1.0940x over previous
"""CrossAttentionBlock on 8 trn2 NeuronCores.

Sharding: data parallel over batch B=2, tensor parallel over heads
(16 heads -> 4 groups of 4). Core c = b*4 + hg.

Key ideas vs the straightforward version:
  - kv compaction: the mask zeroes ~50% of kv positions, and masked softmax
    over the full sequence is EXACTLY softmax over the unmasked subset. The
    host gathers unmasked kv rows (pad to a multiple of 512), halving K/V
    projections, scores, exp and attn@V on device. Padded rows carry a 0
    mask column so they drop out of the denominator; their V rows are 0.
  - fp8 (e4m3) operands everywhere on the PE: inputs/weights are cast on
    the host (pure dtype marshaling), intermediates (q/k/v/exp) are cast
    for free during PSUM evacuation. Projections and attn@V run in
    DoubleRow perf mode (2 contraction rows/cycle).
  - attn@V flipped: out[q(128), d] instead of out[d, q(512)] -> 65-row
    moving operand per call at 0.5 cycles/row.
  - exp computed as exp(s/8 - 2): keeps values <= ~30, inside e4m3 range;
    numerator and denominator scale identically so the ratio is unchanged.
  - softmax denominator from a constant 1.0 column appended to V (masked),
    so no row-max / no bias pass is needed (scores are N(0,1)-scale).

Phase A output: normalized attention [SQ, 256] bf16 per core.
Phase B: rows sharded (256 rows of B*SQ each): out = attn @ Wo.T + bo +
residual, then LayerNorm.
"""

import math
import numpy as np
from contextlib import ExitStack

import ml_dtypes

import concourse.bacc as bacc
import concourse.tile as tile
import concourse.mybir as mybir
from concourse.bass_utils import run_bass_kernel_spmd

F32 = mybir.dt.float32
F32R = mybir.dt.float32r
BF16 = mybir.dt.bfloat16
FP8 = mybir.dt.float8e4
NP_FP8 = ml_dtypes.float8_e4m3
NP_BF16 = ml_dtypes.bfloat16
AF = mybir.ActivationFunctionType
ALU = mybir.AluOpType
AX = mybir.AxisListType
DR = mybir.MatmulPerfMode.DoubleRow

B, SQ, SKV, E = 2, 1024, 4096, 1024
H, D = 16, 64
HG = 4                 # heads per core
HD = HG * D            # 256
P = 128
NE = E // P            # 8
LN_EPS = 1e-5
SCALE = 1.0 / np.sqrt(D)
EXP_SHIFT = -2.0       # exp(s*SCALE + EXP_SHIFT): keeps e4m3 in range

_CACHE = {}


def _build_phase_a(KVP):
    NKV = KVP // P
    nc = bacc.Bacc("TRN2", target_bir_lowering=False, debug=False, num_devices=8)

    qT_d = nc.dram_tensor("qT", [E, SQ], FP8, kind="ExternalInput")
    kvT_d = nc.dram_tensor("kvT", [E, KVP], FP8, kind="ExternalInput")
    wqT_d = nc.dram_tensor("wqT", [P, NE * HD], FP8, kind="ExternalInput")
    wkT_d = nc.dram_tensor("wkT", [P, NE * HD], FP8, kind="ExternalInput")
    wvT_d = nc.dram_tensor("wvT", [P, NE * HD], FP8, kind="ExternalInput")
    bqk_d = nc.dram_tensor("bqkT", [P, 4], F32, kind="ExternalInput")
    bv_d = nc.dram_tensor("bv", [1, HD], F32, kind="ExternalInput")
    mask_d = nc.dram_tensor("mask01", [P, NKV], F32, kind="ExternalInput")
    attn_d = nc.dram_tensor("attn", [SQ, HD], BF16, kind="ExternalOutput")

    with tile.TileContext(nc) as tc, ExitStack() as ctx:
        const = ctx.enter_context(tc.tile_pool(name="const", bufs=1))

        wq_sb = const.tile([P, NE, HD], FP8)
        nc.sync.dma_start(wq_sb[:], wqT_d.ap().rearrange("p (j d) -> p j d", d=HD))
        wk_sb = const.tile([P, NE, HD], FP8)
        nc.sync.dma_start(wk_sb[:], wkT_d.ap().rearrange("p (j d) -> p j d", d=HD))
        wv_sb = const.tile([P, NE, HD], FP8)
        nc.sync.dma_start(wv_sb[:], wvT_d.ap().rearrange("p (j d) -> p j d", d=HD))
        bqk_sb = const.tile([P, 4], F32)
        nc.sync.dma_start(bqk_sb[:], bqk_d.ap())
        bv_sb = const.tile([1, HD], F32)
        nc.sync.dma_start(bv_sb[:], bv_d.ap())
        mask_sb = const.tile([P, NKV], F32)
        nc.sync.dma_start(mask_sb[:], mask_d.ap())

        shift_sb = const.tile([P, 1], F32)
        nc.any.memset(shift_sb[:], 0.0)
        nc.vector.tensor_scalar(shift_sb[:], shift_sb[:], EXP_SHIFT, None, op0=ALU.add)

        # bv broadcast to all partitions (K=1 matmul); bv is usually zero but
        # the add is cheap and keeps the kernel general.
        ones32_sb = const.tile([1, P], F32)
        nc.any.memset(ones32_sb[:], 1.0)
        ones_sb = const.tile([1, P], F32R)
        nc.vector.tensor_copy(ones_sb[:], ones32_sb[:])
        bvr_sb = const.tile([1, HD], F32R)
        nc.vector.tensor_copy(bvr_sb[:], bv_sb[:])
        bv_bc = const.tile([P, HD], F32)

        qTs_sb = const.tile([P, 2, SQ], FP8)          # q^T, dims on partitions
        kT_sb = const.tile([P, 2, KVP], FP8)          # k^T, dims on partitions
        v_sb = const.tile([P, NKV, HG * (D + 1)], FP8)  # v rows + mask column
        attn_sb = const.tile([P, SQ // P, HD], BF16)  # output, q on partitions

        # mask columns: v[:, t, 65*h + 64] = mask tile t
        for h in range(HG):
            nc.vector.tensor_copy(
                v_sb[:].rearrange("p t (h u) -> p t h u", u=D + 1)[:, :, h, D],
                mask_sb[:])

        with ExitStack() as r0:
            ck_pool = r0.enter_context(tc.tile_pool(name="ck", bufs=3))
            pj_ps = r0.enter_context(tc.tile_pool(name="pjps", bufs=2, space="PSUM"))
            gb_ps = r0.enter_context(tc.tile_pool(name="gbps", bufs=1, space="PSUM"))

            psb = gb_ps.tile([P, HD], F32, tag="gb", name="bvbc")
            nc.tensor.matmul(psb[:], ones_sb[:], bvr_sb[:], start=True, stop=True)
            nc.scalar.copy(bv_bc[:], psb[:])

            def dr_proj(ps, w_sb, ch, m):
                for j2 in range(NE // 2):
                    nc.tensor.matmul(ps[:], w_sb[:, 2 * j2:2 * j2 + 2, m * P:(m + 1) * P],
                                     ch[:, 2 * j2:2 * j2 + 2, :],
                                     start=(j2 == 0), stop=(j2 == NE // 2 - 1),
                                     perf_mode=DR)

            for c in range(SQ // 512):
                ch = ck_pool.tile([P, NE, 512], FP8, tag="ch", name=f"chq{c}")
                nc.sync.dma_start(ch[:], qT_d.ap().rearrange(
                    "(j p) s -> p j s", p=P)[:, :, c * 512:(c + 1) * 512])
                for m in range(2):
                    ps = pj_ps.tile([P, 512], F32, tag="pj", name=f"qps{c}_{m}")
                    dr_proj(ps, wq_sb, ch, m)
                    nc.vector.tensor_scalar(qTs_sb[:, m, c * 512:(c + 1) * 512],
                                            ps[:], bqk_sb[:, m:m + 1], None, op0=ALU.add)

            for c in range((KVP + 511) // 512):
                w = min(512, KVP - c * 512)
                ch = ck_pool.tile([P, NE, w], FP8, tag="ch", name=f"chkv{c}")
                nc.sync.dma_start(ch[:], kvT_d.ap().rearrange(
                    "(j p) s -> p j s", p=P)[:, :, c * 512:c * 512 + w])
                for m in range(2):
                    ps = pj_ps.tile([P, w], F32, tag="pj", name=f"kps{c}_{m}")
                    dr_proj(ps, wk_sb, ch, m)
                    nc.vector.tensor_scalar(kT_sb[:, m, c * 512:c * 512 + w],
                                            ps[:], bqk_sb[:, 2 + m:3 + m], None, op0=ALU.add)
                for tt in range(w // P):
                    t = 4 * c + tt
                    ps = pj_ps.tile([P, HD], F32, tag="pj", name=f"vps{t}")
                    for j2 in range(NE // 2):
                        nc.tensor.matmul(ps[:], ch[:, 2 * j2:2 * j2 + 2, tt * P:(tt + 1) * P],
                                         wv_sb[:, 2 * j2:2 * j2 + 2, :],
                                         start=(j2 == 0), stop=(j2 == NE // 2 - 1),
                                         perf_mode=DR)
                    nc.vector.tensor_copy(
                        v_sb[:, t, :].rearrange("p (h u) -> p h u", u=D + 1)[:, :, 0:D],
                        ps[:].rearrange("p (h d) -> p h d", d=D))

        # ---- attention: per q-half, accumulate all 4 heads x 4 q-subtiles
        with ExitStack() as r1:
            sc_ps = r1.enter_context(tc.tile_pool(name="scps", bufs=2, space="PSUM"))
            pv_ps = r1.enter_context(tc.tile_pool(name="pvps", bufs=4, space="PSUM"))
            ex_pool = r1.enter_context(tc.tile_pool(name="expool", bufs=8))
            sm_pool = r1.enter_context(tc.tile_pool(name="smpool", bufs=4))

            NPAIR = NKV // 2

            def att_av(qh, pv, i, h, ex):
                for qq in range(4):
                    nc.tensor.matmul(
                        pv[qq][:, h, :],
                        ex[:, 0:2, qq * P:(qq + 1) * P],
                        v_sb[:, 2 * i:2 * i + 2, h * (D + 1):(h + 1) * (D + 1)],
                        start=(i == 0), stop=(i == NPAIR - 1),
                        perf_mode=DR)

            for qh in range(2):
                pv = [pv_ps.tile([P, HG, D + 1], F32, tag="pv", name=f"pv{qh}_{qq}")
                      for qq in range(4)]
                # 3-deep emission skew: attn@V for exp tile n issues behind
                # the scores for n+3, so the in-order PE queue never waits on
                # an exp semaphore that hasn't long since fired.
                pend = []
                for i in range(NPAIR):
                    for h in range(HG):
                        m, doff = h // 2, (h % 2) * 64
                        sc = sc_ps.tile([P, 2, 512], F32, tag="sc", name=f"s{qh}_{i}_{h}")
                        for tt in range(2):
                            t = 2 * i + tt
                            nc.tensor.matmul(
                                sc[:, tt, :],
                                kT_sb[doff:doff + D, m, t * P:(t + 1) * P],
                                qTs_sb[doff:doff + D, m, qh * 512:(qh + 1) * 512],
                                start=True, stop=True)
                        ex = ex_pool.tile([P, 2, 512], FP8, tag="ex", name=f"e{qh}_{i}_{h}")
                        nc.scalar.activation(ex[:], sc[:], AF.Exp,
                                             bias=shift_sb[:], scale=float(SCALE))
                        pend.append((i, h, ex))
                        if len(pend) > 3:
                            att_av(qh, pv, *pend.pop(0))
                for p_ in pend:
                    att_av(qh, pv, *p_)
                for qq in range(4):
                    g = qh * 4 + qq
                    for h in range(HG):
                        rec = sm_pool.tile([P, 1], F32, tag="rec", name=f"rec{g}_{h}")
                        nc.vector.reciprocal(rec[:], pv[qq][:, h, D:D + 1])
                        nc.vector.tensor_scalar(
                            attn_sb[:, g, h * D:(h + 1) * D],
                            pv[qq][:, h, 0:D], rec[:], None, op0=ALU.mult)
                    nc.vector.tensor_tensor(attn_sb[:, g, :], attn_sb[:, g, :],
                                            bv_bc[:], op=ALU.add)

        nc.sync.dma_start(attn_d.ap().rearrange("(g p) d -> p g d", p=P), attn_sb[:])

    nc.compile()
    return nc


def _build_phase_b():
    R = 2 * P   # 256 rows per core
    nc = bacc.Bacc("TRN2", target_bir_lowering=False, debug=False, num_devices=8)

    aT_d = nc.dram_tensor("aT", [E, R], BF16, kind="ExternalInput")
    woT_d = nc.dram_tensor("woT", [E, E], BF16, kind="ExternalInput")
    qn_d = nc.dram_tensor("qn", [R, E], F32, kind="ExternalInput")
    ogb_d = nc.dram_tensor("ogb", [3, E], F32R, kind="ExternalInput")
    y_d = nc.dram_tensor("y", [R, E], F32, kind="ExternalOutput")

    with tile.TileContext(nc) as tc, ExitStack() as ctx:
        const = ctx.enter_context(tc.tile_pool(name="const", bufs=1))
        ogb_sb = const.tile([1, 3, E], F32R)
        nc.sync.dma_start(ogb_sb[:], ogb_d.ap().rearrange("(o r) e -> o r e", o=1))
        aT_sb = const.tile([P, NE, R], BF16)
        nc.sync.dma_start(aT_sb[:], aT_d.ap().rearrange("(j p) r -> p j r", p=P))
        qn_sb = const.tile([P, 2, E], F32)
        nc.sync.dma_start(qn_sb[:], qn_d.ap().rearrange("(m p) e -> p m e", p=P))
        ones32_sb = const.tile([1, P], F32)
        nc.any.memset(ones32_sb[:], 1.0)
        ones_sb = const.tile([1, P], F32R)
        nc.vector.tensor_copy(ones_sb[:], ones32_sb[:])

        gam_bc = const.tile([P, E], F32)
        bet_bc = const.tile([P, E], F32)

        wo_pool = ctx.enter_context(tc.tile_pool(name="wo", bufs=4))
        ps_pool = ctx.enter_context(tc.tile_pool(name="ps", bufs=2, space="PSUM"))
        gb_ps = ctx.enter_context(tc.tile_pool(name="gbps", bufs=2, space="PSUM"))
        sbp = ctx.enter_context(tc.tile_pool(name="sbp", bufs=2))

        # broadcast gamma/beta before the wo loop so the PE queue serves them
        # first and the LN tail never waits on them
        for half in range(2):
            cs = slice(half * 512, (half + 1) * 512)
            psg = gb_ps.tile([P, 512], F32, tag="gb", name=f"gbg{half}")
            nc.tensor.matmul(psg[:], ones_sb[:], ogb_sb[:, 1, cs], start=True, stop=True)
            nc.scalar.copy(gam_bc[:, cs], psg[:])
            psb = gb_ps.tile([P, 512], F32, tag="gb", name=f"gbb{half}")
            nc.tensor.matmul(psb[:], ones_sb[:], ogb_sb[:, 2, cs], start=True, stop=True)
            nc.scalar.copy(bet_bc[:, cs], psb[:])

        ps_tiles = {}
        wo_tiles = {}
        for half in range(2):
            wo = wo_pool.tile([P, 4, E], BF16, tag="wo", name=f"wo{half}")
            nc.sync.dma_start(wo[:], woT_d.ap().rearrange(
                "(j p) e -> p j e", p=P)[:, half * 4:(half + 1) * 4, :])
            wo_tiles[half] = wo
        for k in range(NE):
            wo = wo_tiles[k // 4]
            for mt in range(2):
                if k == 0:
                    ps_tiles[mt] = ps_pool.tile([P, E], F32, tag="o", name=f"o{mt}")
                for nh in range(2):
                    nc.tensor.matmul(ps_tiles[mt][:, nh * 512:(nh + 1) * 512],
                                     aT_sb[:, k, mt * P:(mt + 1) * P],
                                     wo[:, k % 4, nh * 512:(nh + 1) * 512],
                                     start=(k == 0), stop=False)
        for mt in range(2):
            for nh in range(2):
                nc.tensor.matmul(ps_tiles[mt][:, nh * 512:(nh + 1) * 512], ones_sb[:],
                                 ogb_sb[:, 0, nh * 512:(nh + 1) * 512],
                                 start=False, stop=True)

        # residual + LayerNorm, var = E[x^2] - mean^2 so the two reductions
        # run on different engines (DVE reduce, ACT Square+accum) in parallel
        for mt in range(2):
            x = sbp.tile([P, E], F32, tag="x", name=f"x{mt}")
            nc.vector.tensor_tensor(x[:], ps_tiles[mt][:], qn_sb[:, mt, :], op=ALU.add)
            s1 = sbp.tile([P, 1], F32, tag="s1", name=f"s1{mt}")
            nc.vector.reduce_sum(s1[:], x[:], axis=AX.X)
            sq = sbp.tile([P, E], F32, tag="sq", name=f"sq{mt}")
            ssq = sbp.tile([P, 1], F32, tag="ssq", name=f"ssq{mt}")
            nc.scalar.activation(sq[:], x[:], AF.Square, accum_out=ssq[:])
            nm = sbp.tile([P, 1], F32, tag="nm", name=f"nm{mt}")
            nc.vector.tensor_scalar(nm[:], s1[:], -1.0 / E, None, op0=ALU.mult)
            m2 = sbp.tile([P, 1], F32, tag="m2", name=f"m2{mt}")
            nc.vector.tensor_tensor(m2[:], nm[:], nm[:], op=ALU.mult)
            var = sbp.tile([P, 1], F32, tag="var", name=f"var{mt}")
            nc.vector.tensor_scalar(var[:], ssq[:], 1.0 / E, LN_EPS, op0=ALU.mult, op1=ALU.add)
            nc.vector.tensor_tensor(var[:], var[:], m2[:], op=ALU.subtract)
            rv = sbp.tile([P, 1], F32, tag="rv", name=f"rv{mt}")
            nc.vector.reciprocal(rv[:], var[:])
            rstd = sbp.tile([P, 1], F32, tag="rstd", name=f"rstd{mt}")
            nc.scalar.activation(rstd[:], rv[:], AF.Sqrt)
            yn = sbp.tile([P, E], F32, tag="yn", name=f"yn{mt}")
            nc.vector.tensor_scalar(yn[:], x[:], nm[:], rstd[:], op0=ALU.add, op1=ALU.mult)
            yg = sbp.tile([P, E], F32, tag="yg", name=f"yg{mt}")
            nc.vector.tensor_tensor(yg[:], yn[:], gam_bc[:], op=ALU.mult)
            yb = sbp.tile([P, E], F32, tag="yb", name=f"yb{mt}")
            nc.vector.tensor_tensor(yb[:], yg[:], bet_bc[:], op=ALU.add)
            nc.sync.dma_start(y_d.ap().rearrange("(m p) e -> p m e", p=P)[:, mt, :], yb[:])

    nc.compile()
    return nc


def _get(name, KVP=None):
    key = (name, KVP)
    if key not in _CACHE:
        _CACHE[key] = _build_phase_a(KVP) if name == "a" else _build_phase_b()
    return _CACHE[key]


def kernel(query, key_value, key_value_mask, Wq, bq, Wk, bk, Wv, bv, Wo, bo,
           ln_gamma, ln_beta):
    f = lambda a: np.ascontiguousarray(np.asarray(a, dtype=np.float32))
    f8 = lambda a: np.ascontiguousarray(np.asarray(a).astype(NP_FP8))
    fb = lambda a: np.ascontiguousarray(np.asarray(a).astype(NP_BF16))
    query, key_value = f(query), f(key_value)
    Wq, Wk, Wv, Wo = f(Wq), f(Wk), f(Wv), f(Wo)
    bq, bk, bv, bo = f(bq), f(bk), f(bv), f(bo)
    ln_gamma, ln_beta = f(ln_gamma), f(ln_beta)
    maskb = np.asarray(key_value_mask) != 0

    # compact kv: softmax over masked scores == softmax over unmasked subset
    cnt = [int(maskb[b].sum()) for b in range(B)]
    KVP = max(512, int(256 * math.ceil(max(cnt) / 256)))
    NKV = KVP // P
    kvc = np.zeros((B, KVP, E), np.float32)
    mask01 = np.zeros((B, KVP), np.float32)
    for b in range(B):
        kvc[b, :cnt[b]] = key_value[b][maskb[b]]
        mask01[b, :cnt[b]] = 1.0

    def shuf(w):
        # [p, j*256+d] = W.T[j*128+p, d] -> contiguous 1KB DMA rows
        return np.ascontiguousarray(
            w.T.reshape(NE, P, HD).transpose(1, 0, 2).reshape(P, NE * HD)).astype(NP_FP8)

    nc_a = _get("a", KVP)
    in_maps_a = []
    for c in range(8):
        b, hg = c // 4, c % 4
        sl = slice(hg * HD, (hg + 1) * HD)
        in_maps_a.append({
            "qT": f8(query[b].T),
            "kvT": f8(kvc[b].T),
            "wqT": shuf(Wq[sl]),
            "wkT": shuf(Wk[sl]),
            "wvT": shuf(Wv[sl]),
            "bqkT": f(np.concatenate([bq[sl].reshape(2, P).T,
                                      bk[sl].reshape(2, P).T], axis=1)),
            "bv": bv[sl].reshape(1, HD),
            "mask01": f(mask01[b].reshape(NKV, P).T),
        })
    res_a = run_bass_kernel_spmd(nc_a, in_maps_a, core_ids=list(range(8)))
    # per batch: [SQ, E] bf16 attention output (concat head groups on dims)
    attn = [np.concatenate([res_a.results[b * 4 + hg]["attn"] for hg in range(4)], axis=1)
            for b in range(B)]

    nc_b = _get("b")
    woT = fb(Wo.T)
    ogb = f(np.stack([bo, ln_gamma, ln_beta]))
    in_maps_b = []
    for c in range(8):
        b, j = c // 4, c % 4
        rs = slice(j * 256, (j + 1) * 256)
        in_maps_b.append({
            "aT": np.ascontiguousarray(attn[b].T[:, rs]),
            "woT": woT,
            "qn": f(query[b, rs, :]),
            "ogb": ogb,
        })
    res_b = run_bass_kernel_spmd(nc_b, in_maps_b, core_ids=list(range(8)))
    out = np.empty((B, SQ, E), np.float32)
    for c in range(8):
        b, j = c // 4, c % 4
        out[b, j * 256:(j + 1) * 256, :] = res_b.results[c]["y"]
    return out


# revision 39
# speedup vs baseline: 1.9951x; 1.1064x over previous
"""CrossAttentionBlock on 8 trn2 NeuronCores.

Sharding: data parallel over batch B=2, tensor parallel over heads
(16 heads -> 4 groups of 4). Core c = b*4 + hg.

Key ideas vs the straightforward version:
  - kv compaction: the mask zeroes ~50% of kv positions, and masked softmax
    over the full sequence is EXACTLY softmax over the unmasked subset. The
    host gathers unmasked kv rows (pad to a multiple of 512), halving K/V
    projections, scores, exp and attn@V on device. Padded rows carry a 0
    mask column so they drop out of the denominator; their V rows are 0.
  - fp8 (e4m3) operands everywhere on the PE: inputs/weights are cast on
    the host (pure dtype marshaling), intermediates (q/k/v/exp) are cast
    for free during PSUM evacuation. Projections and attn@V run in
    DoubleRow perf mode (2 contraction rows/cycle).
  - attn@V flipped: out[q(128), d] instead of out[d, q(512)] -> 65-row
    moving operand per call at 0.5 cycles/row.
  - exp computed as exp(s/8 - 2): keeps values <= ~30, inside e4m3 range;
    numerator and denominator scale identically so the ratio is unchanged.
  - softmax denominator from a constant 1.0 column appended to V (masked),
    so no row-max / no bias pass is needed (scores are N(0,1)-scale).

Phase A output: normalized attention [SQ, 256] bf16 per core.
Phase B: rows sharded (256 rows of B*SQ each): out = attn @ Wo.T + bo +
residual, then LayerNorm.
"""

import math
import numpy as np
from contextlib import ExitStack

import ml_dtypes

import concourse.bacc as bacc
import concourse.tile as tile
import concourse.mybir as mybir
from concourse.bass_utils import run_bass_kernel_spmd

F32 = mybir.dt.float32
F32R = mybir.dt.float32r
BF16 = mybir.dt.bfloat16
FP8 = mybir.dt.float8e4
NP_FP8 = ml_dtypes.float8_e4m3
NP_BF16 = ml_dtypes.bfloat16
AF = mybir.ActivationFunctionType
ALU = mybir.AluOpType
AX = mybir.AxisListType
DR = mybir.MatmulPerfMode.DoubleRow

B, SQ, SKV, E = 2, 1024, 4096, 1024
H, D = 16, 64
HG = 4                 # heads per core
HD = HG * D            # 256
P = 128
NE = E // P            # 8
LN_EPS = 1e-5
SCALE = 1.0 / np.sqrt(D)
EXP_SHIFT = -2.0       # exp(s*SCALE + EXP_SHIFT): keeps e4m3 in range

_CACHE = {}


def _build_phase_a(KVP):
    NKV = KVP // P
    nc = bacc.Bacc("TRN2", target_bir_lowering=False, debug=False, num_devices=8)

    qT_d = nc.dram_tensor("qT", [E, SQ], FP8, kind="ExternalInput")
    kvT_d = nc.dram_tensor("kvT", [E, KVP], FP8, kind="ExternalInput")
    wqT_d = nc.dram_tensor("wqT", [P, NE * HD], FP8, kind="ExternalInput")
    wkT_d = nc.dram_tensor("wkT", [P, NE * HD], FP8, kind="ExternalInput")
    wvT_d = nc.dram_tensor("wvT", [P, NE * HD], FP8, kind="ExternalInput")
    bqk_d = nc.dram_tensor("bqkT", [P, 4], F32, kind="ExternalInput")
    bv_d = nc.dram_tensor("bv", [1, HD], F32, kind="ExternalInput")
    mask_d = nc.dram_tensor("mask01", [P, NKV], F32, kind="ExternalInput")
    attn_d = nc.dram_tensor("attn", [SQ, HD], BF16, kind="ExternalOutput")

    with tile.TileContext(nc) as tc, ExitStack() as ctx:
        const = ctx.enter_context(tc.tile_pool(name="const", bufs=1))

        # q-projection inputs issue first so the PE starts ~2us in; the
        # k/v weights follow (emitted just before the kv loop below)
        wq_sb = const.tile([P, NE, HD], FP8)
        nc.sync.dma_start(wq_sb[:], wqT_d.ap().rearrange("p (j d) -> p j d", d=HD))
        bqk_sb = const.tile([P, 4], F32)
        nc.sync.dma_start(bqk_sb[:], bqk_d.ap())
        wk_sb = const.tile([P, NE, HD], FP8)
        wv_sb = const.tile([P, NE, HD], FP8)
        bv_sb = const.tile([1, HD], F32)
        mask_sb = const.tile([P, NKV], F32)

        shift_sb = const.tile([P, 1], F32)
        nc.any.memset(shift_sb[:], 0.0)
        nc.vector.tensor_scalar(shift_sb[:], shift_sb[:], EXP_SHIFT, None, op0=ALU.add)

        # bv broadcast to all partitions (K=1 matmul); bv is usually zero but
        # the add is cheap and keeps the kernel general.
        ones32_sb = const.tile([1, P], F32)
        nc.any.memset(ones32_sb[:], 1.0)
        ones_sb = const.tile([1, P], F32R)
        nc.vector.tensor_copy(ones_sb[:], ones32_sb[:])
        bvr_sb = const.tile([1, HD], F32R)
        bv_bc = const.tile([P, HD], F32)

        qTs_sb = const.tile([P, 2, SQ], FP8)          # q^T, dims on partitions
        kT_sb = const.tile([P, 2, KVP], FP8)          # k^T, dims on partitions
        v_sb = const.tile([P, NKV, HG * (D + 1)], FP8)  # v rows + mask column
        attn_sb = const.tile([P, SQ // P, HD], BF16)  # output, q on partitions

        with ExitStack() as r0:
            ck_pool = r0.enter_context(tc.tile_pool(name="ck", bufs=3))
            pj_ps = r0.enter_context(tc.tile_pool(name="pjps", bufs=2, space="PSUM"))
            gb_ps = r0.enter_context(tc.tile_pool(name="gbps", bufs=1, space="PSUM"))

            def dr_proj(ps, w_sb, ch, m):
                for j2 in range(NE // 2):
                    nc.tensor.matmul(ps[:], w_sb[:, 2 * j2:2 * j2 + 2, m * P:(m + 1) * P],
                                     ch[:, 2 * j2:2 * j2 + 2, :],
                                     start=(j2 == 0), stop=(j2 == NE // 2 - 1),
                                     perf_mode=DR)

            for c in range(SQ // 512):
                ch = ck_pool.tile([P, NE, 512], FP8, tag="ch", name=f"chq{c}")
                nc.sync.dma_start(ch[:], qT_d.ap().rearrange(
                    "(j p) s -> p j s", p=P)[:, :, c * 512:(c + 1) * 512])
                for m in range(2):
                    ps = pj_ps.tile([P, 512], F32, tag="pj", name=f"qps{c}_{m}")
                    dr_proj(ps, wq_sb, ch, m)
                    nc.vector.tensor_scalar(qTs_sb[:, m, c * 512:(c + 1) * 512],
                                            ps[:], bqk_sb[:, m:m + 1], None, op0=ALU.add)

            nc.sync.dma_start(wk_sb[:], wkT_d.ap().rearrange("p (j d) -> p j d", d=HD))
            nc.sync.dma_start(wv_sb[:], wvT_d.ap().rearrange("p (j d) -> p j d", d=HD))
            nc.sync.dma_start(bv_sb[:], bv_d.ap())
            nc.sync.dma_start(mask_sb[:], mask_d.ap())
            nc.vector.tensor_copy(bvr_sb[:], bv_sb[:])
            psb = gb_ps.tile([P, HD], F32, tag="gb", name="bvbc")
            nc.tensor.matmul(psb[:], ones_sb[:], bvr_sb[:], start=True, stop=True)
            nc.scalar.copy(bv_bc[:], psb[:])
            # mask columns: v[:, t, 65*h + 64] = mask tile t
            for h in range(HG):
                nc.vector.tensor_copy(
                    v_sb[:].rearrange("p t (h u) -> p t h u", u=D + 1)[:, :, h, D],
                    mask_sb[:])

            for c in range((KVP + 511) // 512):
                w = min(512, KVP - c * 512)
                ch = ck_pool.tile([P, NE, w], FP8, tag="ch", name=f"chkv{c}")
                nc.sync.dma_start(ch[:], kvT_d.ap().rearrange(
                    "(j p) s -> p j s", p=P)[:, :, c * 512:c * 512 + w])
                for m in range(2):
                    ps = pj_ps.tile([P, w], F32, tag="pj", name=f"kps{c}_{m}")
                    dr_proj(ps, wk_sb, ch, m)
                    nc.vector.tensor_scalar(kT_sb[:, m, c * 512:c * 512 + w],
                                            ps[:], bqk_sb[:, 2 + m:3 + m], None, op0=ALU.add)
                for tt in range(w // P):
                    t = 4 * c + tt
                    ps = pj_ps.tile([P, HD], F32, tag="pj", name=f"vps{t}")
                    for j2 in range(NE // 2):
                        nc.tensor.matmul(ps[:], ch[:, 2 * j2:2 * j2 + 2, tt * P:(tt + 1) * P],
                                         wv_sb[:, 2 * j2:2 * j2 + 2, :],
                                         start=(j2 == 0), stop=(j2 == NE // 2 - 1),
                                         perf_mode=DR)
                    nc.vector.tensor_copy(
                        v_sb[:, t, :].rearrange("p (h u) -> p h u", u=D + 1)[:, :, 0:D],
                        ps[:].rearrange("p (h d) -> p h d", d=D))

        # ---- attention: per q-half, accumulate all 4 heads x 4 q-subtiles
        with ExitStack() as r1:
            sc_ps = r1.enter_context(tc.tile_pool(name="scps", bufs=2, space="PSUM"))
            pv_ps = r1.enter_context(tc.tile_pool(name="pvps", bufs=4, space="PSUM"))
            ex_pool = r1.enter_context(tc.tile_pool(name="expool", bufs=8))
            sm_pool = r1.enter_context(tc.tile_pool(name="smpool", bufs=4))

            NPAIR = NKV // 2

            def att_av(qh, pv, i, h, ex):
                for qq in range(4):
                    nc.tensor.matmul(
                        pv[qq][:, h, :],
                        ex[:, 0:2, qq * P:(qq + 1) * P],
                        v_sb[:, 2 * i:2 * i + 2, h * (D + 1):(h + 1) * (D + 1)],
                        start=(i == 0), stop=(i == NPAIR - 1),
                        perf_mode=DR)

            for qh in range(2):
                pv = [pv_ps.tile([P, HG, D + 1], F32, tag="pv", name=f"pv{qh}_{qq}")
                      for qq in range(4)]
                # 3-deep emission skew: attn@V for exp tile n issues behind
                # the scores for n+3, so the in-order PE queue never waits on
                # an exp semaphore that hasn't long since fired.
                pend = []
                for i in range(NPAIR):
                    for h in range(HG):
                        m, doff = h // 2, (h % 2) * 64
                        sc = sc_ps.tile([P, 2, 512], F32, tag="sc", name=f"s{qh}_{i}_{h}")
                        for tt in range(2):
                            t = 2 * i + tt
                            nc.tensor.matmul(
                                sc[:, tt, :],
                                kT_sb[doff:doff + D, m, t * P:(t + 1) * P],
                                qTs_sb[doff:doff + D, m, qh * 512:(qh + 1) * 512],
                                start=True, stop=True)
                        ex = ex_pool.tile([P, 2, 512], FP8, tag="ex", name=f"e{qh}_{i}_{h}")
                        nc.scalar.activation(ex[:], sc[:], AF.Exp,
                                             bias=shift_sb[:], scale=float(SCALE))
                        pend.append((i, h, ex))
                        if len(pend) > 3:
                            att_av(qh, pv, *pend.pop(0))
                for p_ in pend:
                    att_av(qh, pv, *p_)
                for qq in range(4):
                    g = qh * 4 + qq
                    for h in range(HG):
                        rec = sm_pool.tile([P, 1], F32, tag="rec", name=f"rec{g}_{h}")
                        nc.vector.reciprocal(rec[:], pv[qq][:, h, D:D + 1])
                        nc.vector.tensor_scalar(
                            attn_sb[:, g, h * D:(h + 1) * D],
                            pv[qq][:, h, 0:D], rec[:], None, op0=ALU.mult)
                    nc.vector.tensor_tensor(attn_sb[:, g, :], attn_sb[:, g, :],
                                            bv_bc[:], op=ALU.add)

        nc.sync.dma_start(attn_d.ap().rearrange("(g p) d -> p g d", p=P), attn_sb[:])

    nc.compile()
    return nc


def _build_phase_b():
    R = 2 * P   # 256 rows per core
    nc = bacc.Bacc("TRN2", target_bir_lowering=False, debug=False, num_devices=8)

    aT_d = nc.dram_tensor("aT", [E, R], BF16, kind="ExternalInput")
    woT_d = nc.dram_tensor("woT", [E, E], BF16, kind="ExternalInput")
    qn_d = nc.dram_tensor("qn", [R, E], F32, kind="ExternalInput")
    ogb_d = nc.dram_tensor("ogb", [3, E], F32R, kind="ExternalInput")
    y_d = nc.dram_tensor("y", [R, E], F32, kind="ExternalOutput")

    with tile.TileContext(nc) as tc, ExitStack() as ctx:
        const = ctx.enter_context(tc.tile_pool(name="const", bufs=1))
        ogb_sb = const.tile([1, 3, E], F32R)
        nc.sync.dma_start(ogb_sb[:], ogb_d.ap().rearrange("(o r) e -> o r e", o=1))
        aT_sb = const.tile([P, NE, R], BF16)
        nc.sync.dma_start(aT_sb[:], aT_d.ap().rearrange("(j p) r -> p j r", p=P))
        qn_sb = const.tile([P, 2, E], F32)
        ones32_sb = const.tile([1, P], F32)
        nc.any.memset(ones32_sb[:], 1.0)
        ones_sb = const.tile([1, P], F32R)
        nc.vector.tensor_copy(ones_sb[:], ones32_sb[:])

        gam_bc = const.tile([P, E], F32)
        bet_bc = const.tile([P, E], F32)
        qnb = const.tile([P, 2, E], F32)

        wo_pool = ctx.enter_context(tc.tile_pool(name="wo", bufs=4))
        ps_pool = ctx.enter_context(tc.tile_pool(name="ps", bufs=2, space="PSUM"))
        gb_ps = ctx.enter_context(tc.tile_pool(name="gbps", bufs=2, space="PSUM"))
        sbp = ctx.enter_context(tc.tile_pool(name="sbp", bufs=2))

        # broadcast bo+gamma/beta first so the LN tail never waits on them;
        # bo is folded into qnb = qn + bo (off the critical path) instead of
        # trailing bias matmuls.
        bo_bc = const.tile([P, E], F32)
        for half in range(2):
            cs = slice(half * 512, (half + 1) * 512)
            pso = gb_ps.tile([P, 512], F32, tag="gb", name=f"gbo{half}")
            nc.tensor.matmul(pso[:], ones_sb[:], ogb_sb[:, 0, cs], start=True, stop=True)
            nc.scalar.copy(bo_bc[:, cs], pso[:])
            psg = gb_ps.tile([P, 512], F32, tag="gb", name=f"gbg{half}")
            nc.tensor.matmul(psg[:], ones_sb[:], ogb_sb[:, 1, cs], start=True, stop=True)
            nc.scalar.copy(gam_bc[:, cs], psg[:])
            psb = gb_ps.tile([P, 512], F32, tag="gb", name=f"gbb{half}")
            nc.tensor.matmul(psb[:], ones_sb[:], ogb_sb[:, 2, cs], start=True, stop=True)
            nc.scalar.copy(bet_bc[:, cs], psb[:])

        wo_tiles = []
        for quad in range(4):
            wo = wo_pool.tile([P, 2, E], BF16, tag="wo", name=f"wo{quad}")
            nc.sync.dma_start(wo[:], woT_d.ap().rearrange(
                "(j p) e -> p j e", p=P)[:, quad * 2:(quad + 1) * 2, :])
            wo_tiles.append(wo)
            if quad == 1:
                nc.sync.dma_start(qn_sb[:], qn_d.ap().rearrange("(m p) e -> p m e", p=P))
                for mt in range(2):
                    nc.vector.tensor_tensor(qnb[:, mt, :], qn_sb[:, mt, :],
                                            bo_bc[:], op=ALU.add)

        # per row-tile: matmuls then LayerNorm, so mt=0's LN overlaps mt=1's
        # matmuls. var = E[x^2] - mean^2; sum(x) fused into the residual add.
        for mt in range(2):
            ps = ps_pool.tile([P, E], F32, tag="o", name=f"o{mt}")
            for k in range(NE):
                for nh in range(2):
                    nc.tensor.matmul(ps[:, nh * 512:(nh + 1) * 512],
                                     aT_sb[:, k, mt * P:(mt + 1) * P],
                                     wo_tiles[k // 2][:, k % 2, nh * 512:(nh + 1) * 512],
                                     start=(k == 0), stop=(k == NE - 1))
            x = sbp.tile([P, E], F32, tag="x", name=f"x{mt}")
            nc.vector.tensor_tensor(x[:], ps[:], qnb[:, mt, :], op=ALU.add)
            s1 = sbp.tile([P, 1], F32, tag="s1", name=f"s1{mt}")
            nc.vector.reduce_sum(s1[:], x[:], axis=AX.X)
            sq = sbp.tile([P, E], F32, tag="sq", name=f"sq{mt}")
            ssq = sbp.tile([P, 1], F32, tag="ssq", name=f"ssq{mt}")
            nc.scalar.activation(sq[:], x[:], AF.Square, accum_out=ssq[:])
            nm = sbp.tile([P, 1], F32, tag="nm", name=f"nm{mt}")
            nc.vector.tensor_scalar(nm[:], s1[:], -1.0 / E, None, op0=ALU.mult)
            m2n = sbp.tile([P, 1], F32, tag="m2n", name=f"m2n{mt}")
            nc.vector.tensor_tensor(m2n[:], nm[:], nm[:], op=ALU.mult)
            var = sbp.tile([P, 1], F32, tag="var", name=f"var{mt}")
            nc.vector.tensor_scalar(var[:], ssq[:], 1.0 / E, LN_EPS, op0=ALU.mult, op1=ALU.add)
            nc.vector.tensor_tensor(var[:], var[:], m2n[:], op=ALU.subtract)
            rv = sbp.tile([P, 1], F32, tag="rv", name=f"rv{mt}")
            nc.vector.reciprocal(rv[:], var[:])
            rstd = sbp.tile([P, 1], F32, tag="rstd", name=f"rstd{mt}")
            nc.scalar.activation(rstd[:], rv[:], AF.Sqrt)
            yn = sbp.tile([P, E], F32, tag="yn", name=f"yn{mt}")
            nc.vector.tensor_scalar(yn[:], x[:], nm[:], rstd[:], op0=ALU.add, op1=ALU.mult)
            yg = sbp.tile([P, E], F32, tag="yg", name=f"yg{mt}")
            nc.vector.tensor_tensor(yg[:], yn[:], gam_bc[:], op=ALU.mult)
            yb = sbp.tile([P, E], F32, tag="yb", name=f"yb{mt}")
            nc.vector.tensor_tensor(yb[:], yg[:], bet_bc[:], op=ALU.add)
            nc.sync.dma_start(y_d.ap().rearrange("(m p) e -> p m e", p=P)[:, mt, :], yb[:])

    nc.compile()
    return nc


def _get(name, KVP=None):
    key = (name, KVP)
    if key not in _CACHE:
        _CACHE[key] = _build_phase_a(KVP) if name == "a" else _build_phase_b()
    return _CACHE[key]


def kernel(query, key_value, key_value_mask, Wq, bq, Wk, bk, Wv, bv, Wo, bo,
           ln_gamma, ln_beta):
    f = lambda a: np.ascontiguousarray(np.asarray(a, dtype=np.float32))
    f8 = lambda a: np.ascontiguousarray(np.asarray(a).astype(NP_FP8))
    fb = lambda a: np.ascontiguousarray(np.asarray(a).astype(NP_BF16))
    query, key_value = f(query), f(key_value)
    Wq, Wk, Wv, Wo = f(Wq), f(Wk), f(Wv), f(Wo)
    bq, bk, bv, bo = f(bq), f(bk), f(bv), f(bo)
    ln_gamma, ln_beta = f(ln_gamma), f(ln_beta)
    maskb = np.asarray(key_value_mask) != 0

    # compact kv: softmax over masked scores == softmax over unmasked subset
    cnt = [int(maskb[b].sum()) for b in range(B)]
    KVP = max(512, int(256 * math.ceil(max(cnt) / 256)))
    NKV = KVP // P
    kvc = np.zeros((B, KVP, E), np.float32)
    mask01 = np.zeros((B, KVP), np.float32)
    for b in range(B):
        kvc[b, :cnt[b]] = key_value[b][maskb[b]]
        mask01[b, :cnt[b]] = 1.0

    def shuf(w):
        # [p, j*256+d] = W.T[j*128+p, d] -> contiguous 1KB DMA rows
        return np.ascontiguousarray(
            w.T.reshape(NE, P, HD).transpose(1, 0, 2).reshape(P, NE * HD)).astype(NP_FP8)

    nc_a = _get("a", KVP)
    in_maps_a = []
    for c in range(8):
        b, hg = c // 4, c % 4
        sl = slice(hg * HD, (hg + 1) * HD)
        in_maps_a.append({
            "qT": f8(query[b].T),
            "kvT": f8(kvc[b].T),
            "wqT": shuf(Wq[sl]),
            "wkT": shuf(Wk[sl]),
            "wvT": shuf(Wv[sl]),
            "bqkT": f(np.concatenate([bq[sl].reshape(2, P).T,
                                      bk[sl].reshape(2, P).T], axis=1)),
            "bv": bv[sl].reshape(1, HD),
            "mask01": f(mask01[b].reshape(NKV, P).T),
        })
    res_a = run_bass_kernel_spmd(nc_a, in_maps_a, core_ids=list(range(8)))
    # per batch: [SQ, E] bf16 attention output (concat head groups on dims)
    attn = [np.concatenate([res_a.results[b * 4 + hg]["attn"] for hg in range(4)], axis=1)
            for b in range(B)]

    nc_b = _get("b")
    woT = fb(Wo.T)
    ogb = f(np.stack([bo, ln_gamma, ln_beta]))
    in_maps_b = []
    for c in range(8):
        b, j = c // 4, c % 4
        rs = slice(j * 256, (j + 1) * 256)
        in_maps_b.append({
            "aT": np.ascontiguousarray(attn[b].T[:, rs]),
            "woT": woT,
            "qn": f(query[b, rs, :]),
            "ogb": ogb,
        })
    res_b = run_bass_kernel_spmd(nc_b, in_maps_b, core_ids=list(range(8)))
    out = np.empty((B, SQ, E), np.float32)
    for c in range(8):
        b, j = c // 4, c % 4
        out[b, j * 256:(j + 1) * 256, :] = res_b.results[c]["y"]
    return out
